# revision 1
# baseline (speedup 1.0000x reference)
"""Distributed Trainium2 kernel for a dense transformer block.

Problem: x[2,2048,1024] -> LN1 -> MHA(16 heads, masked) -> +res -> LN2 ->
FFN(4096, gelu) -> +res.

Sharding: token-parallel (sequence-parallel) across 8 cores. Core r owns
batch r//4, token rows [(r%4)*512, (r%4)*512+512). LN1/QKV/proj/LN2/FFN are
fully local per token; only K and V are all-gathered (fp16) within each
4-core batch group before attention.

Attention layout: scores are computed transposed, S^T[keys, rows] =
K^T.T @ Q^T, so the exp'd probabilities P^T land directly in the lhsT layout
the AV matmul needs — no transposes in the inner loop. The softmax
denominator comes for free as a ones-column appended to V, and the division
is applied per head during PSUM evacuation.
"""

import sys

sys.path.insert(0, "/opt/trn_rl_repo")

import numpy as np

B, L, D = 2, 2048, 1024
H, HD = 16, 64
FF = 4 * D
N_CORES = 8
T = (B * L) // N_CORES          # 512 tokens per core
G = N_CORES // B                # 4 cores per batch group
SCALE2 = float(HD) ** -0.5      # applied once to scores (= SCALE^2)
EPS = 1e-5

_cache = {}


def _build(cfg):
    import concourse.bass as bass
    from concourse import bacc, mybir
    import concourse.tile as tile
    from concourse.masks import make_identity

    f32 = mybir.dt.float32
    f16 = mybir.dt.float16
    AF = mybir.ActivationFunctionType
    OP = mybir.AluOpType

    TT = T // 128            # 4 token tiles
    DT = D // 128            # 8 dim chunks
    QKF = 2 * D              # q+k features
    KC = L // 128            # 16 key chunks
    FT = FF // 128           # 32 ffn hidden chunks

    nc = bacc.Bacc("TRN2", target_bir_lowering=False, debug=False,
                   num_devices=N_CORES)

    x_in = nc.dram_tensor("x", [T, D], f32, kind="ExternalInput")
    mask_in = nc.dram_tensor("maskT", [L, T], f16, kind="ExternalInput")
    wqk_in = nc.dram_tensor("w_qk", [D, QKF], f16, kind="ExternalInput")
    wv_in = nc.dram_tensor("w_v", [D, D], f16, kind="ExternalInput")
    wp_in = nc.dram_tensor("w_proj", [D, D], f16, kind="ExternalInput")
    w1_in = nc.dram_tensor("w_f1", [D, FF], f16, kind="ExternalInput")
    w2_in = nc.dram_tensor("w_f2", [FF, D], f16, kind="ExternalInput")
    out_t = nc.dram_tensor("out", [T, D], f32, kind="ExternalOutput")

    # optional affine/bias inputs (only declared when non-trivial)
    opt = {}
    if cfg["ln1_affine"]:
        opt["ln1_wb"] = nc.dram_tensor("ln1_wb", [2, D], f32, kind="ExternalInput")
    if cfg["ln2_affine"]:
        opt["ln2_wb"] = nc.dram_tensor("ln2_wb", [2, D], f32, kind="ExternalInput")
    if cfg["qkv_bias"]:
        opt["bqk"] = nc.dram_tensor("bqk", [QKF, 1], f32, kind="ExternalInput")
        opt["bv"] = nc.dram_tensor("bv", [1, D], f32, kind="ExternalInput")
    if cfg["proj_bias"]:
        opt["bproj"] = nc.dram_tensor("bproj", [1, D], f32, kind="ExternalInput")
    if cfg["ffn1_bias"]:
        opt["bf1"] = nc.dram_tensor("bf1", [FF, 1], f32, kind="ExternalInput")
    if cfg["ffn2_bias"]:
        opt["bf2"] = nc.dram_tensor("bf2", [1, D], f32, kind="ExternalInput")

    with tile.TileContext(nc) as tc:
        pp = tc.alloc_tile_pool(name="persist", bufs=1)
        wp = tc.alloc_tile_pool(name="work", bufs=3)
        wtp = tc.alloc_tile_pool(name="wtile", bufs=6)
        pszoo = tc.alloc_tile_pool(name="psums", bufs=2, space="PSUM")
        dp = tc.alloc_tile_pool(name="dram", bufs=1, space="DRAM")

        identity = pp.tile([128, 128], f16, tag="identity", name="identity")
        make_identity(nc, identity[:])
        eps_sb = pp.tile([128, 1], f32, tag="eps", name="eps")
        nc.vector.memset(eps_sb[:], EPS)

        # broadcast helper for optional per-free-dim vectors (dram [1, n])
        def bcast_tile(src_ap, n, tag):
            row = pp.tile([1, n], f32, tag=tag + "r", name=tag + "r")
            nc.sync.dma_start(row[:], src_ap)
            t_ = pp.tile([128, n], f32, tag=tag, name=tag)
            nc.gpsimd.partition_broadcast(t_[:], row[:])
            return t_

        ln1_w_bc = ln1_b_bc = ln2_w_bc = ln2_b_bc = None
        if cfg["ln1_affine"]:
            ln1_w_bc = bcast_tile(opt["ln1_wb"].ap()[0:1, :], D, "ln1w")
            ln1_b_bc = bcast_tile(opt["ln1_wb"].ap()[1:2, :], D, "ln1b")
        if cfg["ln2_affine"]:
            ln2_w_bc = bcast_tile(opt["ln2_wb"].ap()[0:1, :], D, "ln2w")
            ln2_b_bc = bcast_tile(opt["ln2_wb"].ap()[1:2, :], D, "ln2b")
        bv_bc = bcast_tile(opt["bv"].ap(), D, "bv") if cfg["qkv_bias"] else None
        bp_bc = bcast_tile(opt["bproj"].ap(), D, "bp") if cfg["proj_bias"] else None
        bf2_bc = bcast_tile(opt["bf2"].ap(), D, "bf2") if cfg["ffn2_bias"] else None
        bqk_sb = None
        if cfg["qkv_bias"]:
            bqk_sb = [pp.tile([128, 1], f32, tag=f"bqk{f}", name=f"bqk{f}")
                      for f in range(QKF // 128)]
            for f in range(QKF // 128):
                nc.sync.dma_start(bqk_sb[f][:],
                                  opt["bqk"].ap()[f * 128:(f + 1) * 128, :])
        bf1_sb = None
        if cfg["ffn1_bias"]:
            bf1_sb = [pp.tile([128, 1], f32, tag=f"bf1{m}", name=f"bf1{m}")
                      for m in range(FT)]
            for m in range(FT):
                nc.sync.dma_start(bf1_sb[m][:],
                                  opt["bf1"].ap()[m * 128:(m + 1) * 128, :])

        # ---------------- layer norm -> transposed fp16 ----------------
        def layer_norm_T(src_tiles, dstT_tiles, w_bc, b_bc, affine, tag):
            """src: TT tiles [128, D] f32; dst: DT tiles [128, T] f16."""
            for i in range(TT):
                xt = src_tiles[i]
                mu = wp.tile([128, 1], f32, tag="lnmu", name="lnmu")
                nc.vector.tensor_reduce(mu[:], xt[:], mybir.AxisListType.X, OP.add)
                nc.vector.tensor_scalar_mul(mu[:], mu[:], 1.0 / D)
                junk = wp.tile([128, D], f16, tag="lnjunk", name="lnjunk", bufs=1)
                varr = wp.tile([128, 1], f32, tag="lnvar", name="lnvar")
                nc.vector.scalar_tensor_tensor(
                    junk[:], xt[:], mu[:], xt[:],
                    op0=OP.subtract, op1=OP.mult, accum_out=varr[:])
                std = wp.tile([128, 1], f32, tag="lnstd", name="lnstd")
                nc.scalar.activation(std[:], varr[:], AF.Sqrt,
                                     bias=eps_sb[:], scale=1.0 / D)
                rstd = wp.tile([128, 1], f32, tag="lnrstd", name="lnrstd")
                nc.vector.reciprocal(rstd[:], std[:])
                xn = wp.tile([128, D], f16, tag="lnxn", name="lnxn")
                nc.vector.tensor_scalar(xn[:], xt[:], mu[:], rstd[:],
                                        op0=OP.subtract, op1=OP.mult)
                if affine:
                    nc.vector.tensor_tensor(xn[:], xn[:], w_bc[:], op=OP.mult)
                    nc.vector.tensor_tensor(xn[:], xn[:], b_bc[:], op=OP.add)
                for j in range(DT):
                    ps = pszoo.tile([128, 128], f16, tag="tr", name="tr")
                    nc.tensor.transpose(ps[:], xn[:, j * 128:(j + 1) * 128],
                                        identity[:])
                    nc.vector.tensor_copy(
                        dstT_tiles[j][:, i * 128:(i + 1) * 128], ps[:])

        # ---------------- phase A: x load + LN1 ----------------
        qkv_pool = tc.alloc_tile_pool(name="qkvp", bufs=1)
        xnT_pool = tc.alloc_tile_pool(name="xnT", bufs=1)
        wv_pool = tc.alloc_tile_pool(name="wvp", bufs=1)
        x_sb = [pp.tile([128, D], f32, tag=f"x{i}", name=f"x{i}")
                for i in range(TT)]
        for i in range(TT):
            nc.sync.dma_start(x_sb[i][:], x_in.ap()[i * 128:(i + 1) * 128, :])
        xnT = [xnT_pool.tile([128, T], f16, tag=f"xnT{j}", name=f"xnT{j}")
               for j in range(DT)]
        layer_norm_T(x_sb, xnT, ln1_w_bc, ln1_b_bc, cfg["ln1_affine"], "ln1")

        # ---------------- phase B: qkv gemms ----------------
        qkT = [qkv_pool.tile([128, T], f16, tag=f"qkT{f}", name=f"qkT{f}")
               for f in range(QKF // 128)]
        for f in range(QKF // 128):
            ps = pszoo.tile([128, T], f32, tag="acc", name="accqk")
            for j in range(DT):
                wt = wtp.tile([128, 128], f16, tag="wqk", name="wqk")
                nc.sync.dma_start(
                    wt[:], wqk_in.ap()[j * 128:(j + 1) * 128,
                                       f * 128:(f + 1) * 128])
                nc.tensor.matmul(ps[:], wt[:], xnT[j][:],
                                 start=(j == 0), stop=(j == DT - 1))
            if cfg["qkv_bias"]:
                nc.vector.tensor_scalar_add(qkT[f][:], ps[:], bqk_sb[f][:])
            else:
                nc.vector.tensor_copy(qkT[f][:], ps[:])

        # per-head Q^T tiles at base partition 0 (matmul operands must share
        # base partition with kt_h)
        qT_sb = [qkv_pool.tile([64, T], f16, tag=f"qT{h}", name=f"qT{h}")
                 for h in range(H)]
        for h in range(H):
            lo = (h % 2) * 64
            nc.sync.dma_start(qT_sb[h][:], qkT[h // 2][lo:lo + 64, :])

        # v [T, D] normal layout: stationary xnT chunk, moving w_v columns
        wv_sb = [wv_pool.tile([128, 512], f16, tag=f"wv{k}", name=f"wv{k}")
                 for k in range(DT * 2)]
        for j in range(DT):
            for n in range(2):
                nc.sync.dma_start(
                    wv_sb[j * 2 + n][:],
                    wv_in.ap()[j * 128:(j + 1) * 128, n * 512:(n + 1) * 512])
        v_sb = [qkv_pool.tile([128, D], f16, tag=f"v{i}", name=f"v{i}")
                for i in range(TT)]
        for i in range(TT):
            for n in range(2):
                ps = pszoo.tile([128, 512], f32, tag="acc", name="accv")
                for j in range(DT):
                    nc.tensor.matmul(ps[:], xnT[j][:, i * 128:(i + 1) * 128],
                                     wv_sb[j * 2 + n][:],
                                     start=(j == 0), stop=(j == DT - 1))
                dst = v_sb[i][:, n * 512:(n + 1) * 512]
                if cfg["qkv_bias"]:
                    nc.vector.tensor_tensor(dst, ps[:],
                                            bv_bc[:, n * 512:(n + 1) * 512],
                                            op=OP.add)
                else:
                    nc.vector.tensor_copy(dst, ps[:])

        # ---------------- phase C: all-gather K^T and V ----------------
        groups = [list(range(G)), list(range(G, 2 * G))]
        kt_shard = dp.tile([D, T], f16)
        v_shard = dp.tile([T, D], f16)
        for f in range(DT):
            nc.sync.dma_start(kt_shard[f * 128:(f + 1) * 128, :], qkT[DT + f][:])
        for i in range(TT):
            nc.sync.dma_start(v_shard[i * 128:(i + 1) * 128, :], v_sb[i][:])
        kt_g = dp.tile([G * D, T], f16)
        v_g = dp.tile([G * T, D], f16)
        nc.gpsimd.collective_compute(
            "AllGather", OP.bypass, replica_groups=groups,
            ins=[kt_shard[:].opt()], outs=[kt_g[:].opt()])
        nc.gpsimd.collective_compute(
            "AllGather", OP.bypass, replica_groups=groups,
            ins=[v_shard[:].opt()], outs=[v_g[:].opt()])
        wv_pool.release()
        xnT_pool.release()

        # ---------------- phase D: attention ----------------
        attn_pool = tc.alloc_tile_pool(name="attnp", bufs=1)
        mask_pool = tc.alloc_tile_pool(name="maskp", bufs=1)
        mask_sb = [mask_pool.tile([128, T], f16, tag=f"m{c}", name=f"m{c}")
                   for c in range(KC)]
        for c in range(KC):
            nc.sync.dma_start(mask_sb[c][:],
                              mask_in.ap()[c * 128:(c + 1) * 128, :])

        attnT = [attn_pool.tile([128, T], f16, tag=f"aT{j}", name=f"aT{j}")
                 for j in range(DT)]
        v_g_r = v_g[:].rearrange("(c p) n -> p c n", p=128)
        for h in range(H):
            kt_h = wp.tile([64, L], f16, tag="kt_h", name="kt_h", bufs=2)
            for g in range(G):
                nc.sync.dma_start(
                    kt_h[:, g * T:(g + 1) * T],
                    kt_g[g * D + h * HD:g * D + (h + 1) * HD, :])
            vaug = wp.tile([128, KC, HD + 1], f16, tag="vaug", name="vaug",
                           bufs=2)
            nc.vector.memset(vaug[:, :, HD:HD + 1], 1.0)
            nc.sync.dma_start(vaug[:, :, 0:HD],
                              v_g_r[:, :, h * HD:(h + 1) * HD])

            o_ps = pszoo.tile([HD + 1, T], f32, tag="ops", name="ops")
            for c in range(KC):
                s_ps = pszoo.tile([128, T], f32, tag="sps", name="sps")
                nc.tensor.matmul(s_ps[:], kt_h[:, c * 128:(c + 1) * 128],
                                 qT_sb[h][:], start=True, stop=True)
                pt = wp.tile([128, T], f16, tag="pt", name="pt")
                nc.scalar.activation(pt[:], s_ps[:], AF.Exp, scale=SCALE2)
                nc.vector.tensor_tensor(pt[:], pt[:], mask_sb[c][:], op=OP.mult)
                nc.tensor.matmul(o_ps[:], vaug[:, c:c + 1, :], pt[:],
                                 start=(c == 0), stop=(c == KC - 1))
            # divide by the summed column (softmax denominator)
            recip = wp.tile([1, T], f32, tag="recip", name="recip", bufs=2)
            nc.vector.reciprocal(recip[:], o_ps[HD:HD + 1, :])
            rb = wp.tile([64, T], f32, tag="rb", name="rb", bufs=2)
            nc.gpsimd.partition_broadcast(rb[:], recip[:])
            oT_h = wp.tile([64, T], f16, tag="oT_h", name="oT_h", bufs=2)
            nc.vector.tensor_tensor(oT_h[:], o_ps[0:HD, :], rb[:], op=OP.mult)
            lo = (h % 2) * 64
            nc.sync.dma_start(attnT[h // 2][lo:lo + 64, :], oT_h[:])
        mask_pool.release()

        # ---------------- phase E: proj + residual ----------------
        wpp = tc.alloc_tile_pool(name="wpp", bufs=1)
        wproj_sb = [wpp.tile([128, 512], f16, tag=f"wp{k}", name=f"wp{k}")
                    for k in range(DT * 2)]
        for j in range(DT):
            for n in range(2):
                nc.sync.dma_start(
                    wproj_sb[j * 2 + n][:],
                    wp_in.ap()[j * 128:(j + 1) * 128, n * 512:(n + 1) * 512])
        h_sb = [pp.tile([128, D], f32, tag=f"h{i}", name=f"h{i}")
                for i in range(TT)]
        for i in range(TT):
            for n in range(2):
                ps = pszoo.tile([128, 512], f32, tag="acc", name="accp")
                for j in range(DT):
                    nc.tensor.matmul(ps[:], attnT[j][:, i * 128:(i + 1) * 128],
                                     wproj_sb[j * 2 + n][:],
                                     start=(j == 0), stop=(j == DT - 1))
                dst = h_sb[i][:, n * 512:(n + 1) * 512]
                xsl = x_sb[i][:, n * 512:(n + 1) * 512]
                if cfg["proj_bias"]:
                    nc.vector.tensor_tensor(dst, ps[:],
                                            bp_bc[:, n * 512:(n + 1) * 512],
                                            op=OP.add)
                    nc.vector.tensor_tensor(dst, dst, xsl, op=OP.add)
                else:
                    nc.vector.tensor_tensor(dst, ps[:], xsl, op=OP.add)
        wpp.release()
        attn_pool.release()
        qkv_pool.release()

        # ---------------- phase F: LN2 ----------------
        hid_pool = tc.alloc_tile_pool(name="hidp", bufs=1)
        yT_pool = tc.alloc_tile_pool(name="yTp", bufs=1)
        yT = [yT_pool.tile([128, T], f16, tag=f"yT{j}", name=f"yT{j}")
              for j in range(DT)]
        layer_norm_T(h_sb, yT, ln2_w_bc, ln2_b_bc, cfg["ln2_affine"], "ln2")

        # ---------------- phase G: ffn1 + gelu ----------------
        hidT = [hid_pool.tile([128, T], f16, tag=f"hidT{m}", name=f"hidT{m}")
                for m in range(FT)]
        for m in range(FT):
            ps = pszoo.tile([128, T], f32, tag="acc", name="accf1")
            for j in range(DT):
                wt = wtp.tile([128, 128], f16, tag="w1", name="w1")
                nc.sync.dma_start(
                    wt[:], w1_in.ap()[j * 128:(j + 1) * 128,
                                      m * 128:(m + 1) * 128])
                nc.tensor.matmul(ps[:], wt[:], yT[j][:],
                                 start=(j == 0), stop=(j == DT - 1))
            if cfg["ffn1_bias"]:
                nc.scalar.activation(hidT[m][:], ps[:], AF.Gelu,
                                     bias=bf1_sb[m][:])
            else:
                nc.scalar.activation(hidT[m][:], ps[:], AF.Gelu)
        yT_pool.release()

        # ---------------- phase H: ffn2 + residual -> out ----------------
        w2_pool = tc.alloc_tile_pool(name="w2p", bufs=1)
        for n in range(2):
            w2_sb = [w2_pool.tile([128, 512], f16, tag=f"w2_{m}",
                                  name=f"w2_{m}") for m in range(FT)]
            for m in range(FT):
                nc.sync.dma_start(
                    w2_sb[m][:], w2_in.ap()[m * 128:(m + 1) * 128,
                                            n * 512:(n + 1) * 512])
            for i in range(TT):
                ps = pszoo.tile([128, 512], f32, tag="acc", name="accf2")
                for m in range(FT):
                    nc.tensor.matmul(ps[:], hidT[m][:, i * 128:(i + 1) * 128],
                                     w2_sb[m][:],
                                     start=(m == 0), stop=(m == FT - 1))
                o_sb = wp.tile([128, 512], f32, tag="o_sb", name="o_sb")
                hsl = h_sb[i][:, n * 512:(n + 1) * 512]
                if cfg["ffn2_bias"]:
                    nc.vector.tensor_tensor(o_sb[:], ps[:],
                                            bf2_bc[:, n * 512:(n + 1) * 512],
                                            op=OP.add)
                    nc.vector.tensor_tensor(o_sb[:], o_sb[:], hsl, op=OP.add)
                else:
                    nc.vector.tensor_tensor(o_sb[:], ps[:], hsl, op=OP.add)
                nc.sync.dma_start(
                    out_t.ap()[i * 128:(i + 1) * 128,
                               n * 512:(n + 1) * 512], o_sb[:])
        w2_pool.release()
        hid_pool.release()
        wtp.release()
        wp.release()
        pszoo.release()
        dp.release()
        pp.release()

    nc.compile()
    return nc


def _prep(inputs):
    x = np.asarray(inputs["x"], np.float32)
    mask = np.asarray(inputs["mask"])
    qkv_w = np.asarray(inputs["qkv_w"], np.float32)
    qkv_b = np.asarray(inputs["qkv_b"], np.float32)
    proj_w = np.asarray(inputs["proj_w"], np.float32)
    proj_b = np.asarray(inputs["proj_b"], np.float32)
    ffn_w1 = np.asarray(inputs["ffn_w1"], np.float32)
    ffn_b1 = np.asarray(inputs["ffn_b1"], np.float32)
    ffn_w2 = np.asarray(inputs["ffn_w2"], np.float32)
    ffn_b2 = np.asarray(inputs["ffn_b2"], np.float32)
    ln1_w = np.asarray(inputs["ln1_w"], np.float32)
    ln1_b = np.asarray(inputs["ln1_b"], np.float32)
    ln2_w = np.asarray(inputs["ln2_w"], np.float32)
    ln2_b = np.asarray(inputs["ln2_b"], np.float32)

    cfg = {
        "ln1_affine": not (np.allclose(ln1_w, 1.0) and np.allclose(ln1_b, 0.0)),
        "ln2_affine": not (np.allclose(ln2_w, 1.0) and np.allclose(ln2_b, 0.0)),
        "qkv_bias": bool(np.any(qkv_b)),
        "proj_bias": bool(np.any(proj_b)),
        "ffn1_bias": bool(np.any(ffn_b1)),
        "ffn2_bias": bool(np.any(ffn_b2)),
    }

    w_qk = np.ascontiguousarray(qkv_w[:, :2 * D]).astype(np.float16)
    w_v = np.ascontiguousarray(qkv_w[:, 2 * D:]).astype(np.float16)
    w_p16 = proj_w.astype(np.float16)
    w1_16 = ffn_w1.astype(np.float16)
    w2_16 = ffn_w2.astype(np.float16)

    in_maps = []
    for r in range(N_CORES):
        b = r // G
        row0 = (r % G) * T
        im = {
            "x": np.ascontiguousarray(x[b, row0:row0 + T, :]),
            "maskT": np.ascontiguousarray(
                (mask[b, 0, row0:row0 + T, :] != 0).T.astype(np.float16)),
            "w_qk": w_qk, "w_v": w_v, "w_proj": w_p16,
            "w_f1": w1_16, "w_f2": w2_16,
        }
        if cfg["ln1_affine"]:
            im["ln1_wb"] = np.ascontiguousarray(np.stack([ln1_w, ln1_b]))
        if cfg["ln2_affine"]:
            im["ln2_wb"] = np.ascontiguousarray(np.stack([ln2_w, ln2_b]))
        if cfg["qkv_bias"]:
            im["bqk"] = np.ascontiguousarray(qkv_b[:2 * D, None])
            im["bv"] = np.ascontiguousarray(qkv_b[None, 2 * D:])
        if cfg["proj_bias"]:
            im["bproj"] = np.ascontiguousarray(proj_b[None, :])
        if cfg["ffn1_bias"]:
            im["bf1"] = np.ascontiguousarray(ffn_b1[:, None])
        if cfg["ffn2_bias"]:
            im["bf2"] = np.ascontiguousarray(ffn_b2[None, :])
        in_maps.append(im)
    return cfg, in_maps


def _run(inputs, trace=False):
    from concourse.bass_utils import run_bass_kernel_spmd

    cfg, in_maps = _prep(inputs)
    key = tuple(sorted(cfg.items()))
    if key not in _cache:
        _cache[key] = _build(cfg)
    nc = _cache[key]
    res = run_bass_kernel_spmd(nc, in_maps, core_ids=list(range(N_CORES)),
                               trace=trace)
    out = np.empty((B, L, D), np.float32)
    for r in range(N_CORES):
        b = r // G
        row0 = (r % G) * T
        out[b, row0:row0 + T, :] = res.results[r]["out"]
    return out, res


def kernel(**inputs):
    out, _ = _run(inputs, trace=False)
    return out



# revision 13
# speedup vs baseline: 1.5993x; 1.5993x over previous
"""Distributed Trainium2 kernel for a dense transformer block.

Problem: x[2,2048,1024] -> LN1 -> MHA(16 heads, causal) -> +res -> LN2 ->
FFN(4096, gelu) -> +res.

Fast path (mask verified causal-tril on host, biases zero, LN affine
identity): head-parallel attention. Core r (batch b=r//4, rank j=r%4) owns
heads 4j..4j+3 for all 2048 queries of its batch, and token strips
{qb*512 + j*128 .. +128} for the token-parallel parts (LN/FFN/residual).

- LN1 runs sequence-parallel on own 512 tokens; xn^T is all-gathered in fp8
  (one small collective instead of two fp16 K/V gathers).
- QKV/proj gemms run in fp8 with DoubleRow (2 contraction rows per
  partition); scores in plain fp8 (K=64); AV in fp8 DoubleRow with a ones
  column appended to V so the softmax denominator accumulates for free.
- Causality is exploited uniformly across cores: query block qb only
  touches key chunks 0..4qb+3, so scores/exp/AV work halves. Only the
  diagonal 512x512 block is masked (with real mask data).
- proj partials are reduce-scattered per query block (4 pipelined fp16
  collectives, descending qb so the last one is the cheapest block).
- FFN stays fp16 (fp8 fails the tolerance there), with wide weight DMAs.

Generic fallback (non-causal mask or non-trivial biases): the previous
token-parallel kernel, kept verbatim below.
"""

import sys

sys.path.insert(0, "/opt/trn_rl_repo")

import numpy as np

B, L, D = 2, 2048, 1024
H, HD = 16, 64
FF = 4 * D
N_CORES = 8
G = N_CORES // B                # 4 cores per batch group
T = (B * L) // N_CORES          # 512 tokens per core
HPC = H // G                    # 4 heads per core
NQB = L // 512                  # 4 query blocks
KC = L // 128                   # 16 key chunks
SCALE2 = float(HD) ** -0.5
EPS = 1e-5

_cache = {}


# ------------------------------------------------------------------
# fast path: head-parallel causal kernel
# ------------------------------------------------------------------
def _build_fast():
    import concourse.bass as bass
    from concourse import bacc, mybir
    import concourse.tile as tile
    from concourse.masks import make_identity

    f32 = mybir.dt.float32
    f16 = mybir.dt.float16
    f8 = mybir.dt.float8e4
    AF = mybir.ActivationFunctionType
    OP = mybir.AluOpType
    DR = mybir.MatmulPerfMode.DoubleRow

    DT = D // 128               # 8 dim chunks
    FT = FF // 128              # 32 ffn hidden chunks

    nc = bacc.Bacc("TRN2", target_bir_lowering=False, debug=False,
                   num_devices=N_CORES)

    x_in = nc.dram_tensor("x", [T, D], f32, kind="ExternalInput")
    maskd_in = nc.dram_tensor("maskd", [NQB * 512, 512], f8,
                              kind="ExternalInput")
    wqkv_in = nc.dram_tensor("wqkv", [D, 3 * HPC * HD], f8,
                             kind="ExternalInput")
    wp_in = nc.dram_tensor("wp", [HPC * HD, D], f8, kind="ExternalInput")
    w1_in = nc.dram_tensor("w_f1", [D, FF], f16, kind="ExternalInput")
    w2_in = nc.dram_tensor("w_f2", [FF, D], f16, kind="ExternalInput")
    out_t = nc.dram_tensor("out", [T, D], f32, kind="ExternalOutput")

    groups = [list(range(G)), list(range(G, 2 * G))]

    with tile.TileContext(nc) as tc:
        pp = tc.alloc_tile_pool(name="persist", bufs=1)
        wrk = tc.alloc_tile_pool(name="work", bufs=2)
        dp = tc.alloc_tile_pool(name="dram", bufs=1, space="DRAM")

        identity = pp.tile([128, 128], f16, tag="identity", name="identity")
        make_identity(nc, identity[:])
        eps_sb = pp.tile([128, 1], f32, tag="eps", name="eps")
        nc.vector.memset(eps_sb[:], EPS)

        # ---------- phase A: x load + LN1 + transpose to fp8 shard ----------
        psA = tc.alloc_tile_pool(name="psA", bufs=2, space="PSUM")
        xpool = tc.alloc_tile_pool(name="xp", bufs=1, side="right")
        shp = tc.alloc_tile_pool(name="shp", bufs=1, side="right")

        x_sb = [xpool.tile([128, D], f32, tag=f"x{i}", name=f"x{i}")
                for i in range(4)]
        for i in range(4):
            nc.sync.dma_start(x_sb[i][:], x_in.ap()[i * 128:(i + 1) * 128, :])

        def layer_norm_tile(xt, tag):
            """xt [128, D] f32 -> normalized [128, D] f16."""
            mu = wrk.tile([128, 1], f32, tag="lnmu", name="lnmu")
            nc.vector.tensor_reduce(mu[:], xt[:], mybir.AxisListType.X, OP.add)
            nc.vector.tensor_scalar_mul(mu[:], mu[:], 1.0 / D)
            junk = wrk.tile([128, D], f16, tag="lnjunk", name="lnjunk", bufs=1)
            varr = wrk.tile([128, 1], f32, tag="lnvar", name="lnvar")
            nc.vector.scalar_tensor_tensor(
                junk[:], xt[:], mu[:], xt[:],
                op0=OP.subtract, op1=OP.mult, accum_out=varr[:])
            std = wrk.tile([128, 1], f32, tag="lnstd", name="lnstd")
            nc.scalar.activation(std[:], varr[:], AF.Sqrt,
                                 bias=eps_sb[:], scale=1.0 / D)
            rstd = wrk.tile([128, 1], f32, tag="lnrstd", name="lnrstd")
            nc.vector.reciprocal(rstd[:], std[:])
            xn = wrk.tile([128, D], f16, tag="lnxn", name="lnxn")
            nc.vector.tensor_scalar(xn[:], xt[:], mu[:], rstd[:],
                                    op0=OP.subtract, op1=OP.mult)
            return xn

        xsh_sb = shp.tile([128, DT, T], f8, tag="xsh", name="xsh")
        for i in range(4):
            xn = layer_norm_tile(x_sb[i], f"ln1_{i}")
            for dc in range(DT):
                ps = psA.tile([128, 128], f16, tag="tr", name="trA")
                nc.tensor.transpose(ps[:], xn[:, dc * 128:(dc + 1) * 128],
                                    identity[:])
                nc.vector.tensor_copy(
                    xsh_sb[:, dc, i * 128:(i + 1) * 128], ps[:])

        xshard = dp.tile([D, T], f8)
        nc.sync.dma_start(
            xshard[:].rearrange("(c p) t -> p c t", p=128), xsh_sb[:])
        xg = dp.tile([G * D, T], f8)
        nc.gpsimd.collective_compute(
            "AllGather", OP.bypass, replica_groups=groups,
            ins=[xshard[:].opt()], outs=[xg[:].opt()])

        # weights for qkv/proj/mask arrive during LN1/AG
        wqp = tc.alloc_tile_pool(name="wqp", bufs=1, side="right")
        wqkv_sb = wqp.tile([128, DT, 3 * HPC * HD], f8, tag="wqkv",
                           name="wqkv")
        for k in range(DT):
            nc.sync.dma_start(wqkv_sb[:, k, :],
                              wqkv_in.ap()[k * 128:(k + 1) * 128, :])
        attp = tc.alloc_tile_pool(name="attp", bufs=1, side="right")
        wp_sb = attp.tile([128, 2, D], f8, tag="wp", name="wp")
        for k in range(2):
            nc.sync.dma_start(wp_sb[:, k, :],
                              wp_in.ap()[k * 128:(k + 1) * 128, :])
        maskd_sb = attp.tile([128, 4 * NQB, 512], f8, tag="maskd",
                             name="maskd")
        nc.sync.dma_start(
            maskd_sb[:], maskd_in.ap().rearrange("(g p) q -> p g q", p=128))

        # gathered xn^T -> [128, DT, L]
        xnp = tc.alloc_tile_pool(name="xnp", bufs=1, side="right")
        xnT = xnp.tile([128, DT, L], f8, tag="xnT", name="xnT")
        for g in range(G):
            nc.sync.dma_start(
                xnT[:, :, g * T:(g + 1) * T],
                xg[g * D:(g + 1) * D, :].rearrange("(c p) t -> p c t", p=128))

        # ---------- phase C: QKV gemms (fp8 DoubleRow) ----------
        qT = [attp.tile([128, L], f8, tag=f"qT{p}", name=f"qT{p}")
              for p in range(2)]
        kt = [attp.tile([128, L], f8, tag=f"kt{p}", name=f"kt{p}")
              for p in range(2)]
        # vaug[:, kc, h, 0:64] = V[kc*128+p, h*64+v]; [..., 64] = 1.0
        # padded to 68 so the DoubleRow k-pair stride (HPC*68 B) is 16B-aligned
        VP = 68
        vaug = attp.tile([128, KC, HPC, VP], f8, tag="vaug", name="vaug")
        for kc in range(KC):
            nc.vector.memset(vaug[:, kc, :, HD:HD + 1], 1.0)

        for p in range(2):
            for blk in range(NQB):
                for which, dst in ((0, qT[p]), (1, kt[p])):
                    col0 = which * HPC * HD + p * 128
                    ps = psA.tile([128, 512], f32, tag="qk", name="qkps")
                    for k2 in range(DT // 2):
                        nc.tensor.matmul(
                            ps[:],
                            wqkv_sb[:, 2 * k2:2 * k2 + 2, col0:col0 + 128],
                            xnT[:, 2 * k2:2 * k2 + 2,
                                blk * 512:(blk + 1) * 512],
                            start=(k2 == 0), stop=(k2 == DT // 2 - 1),
                            perf_mode=DR)
                    nc.vector.tensor_copy(dst[:, blk * 512:(blk + 1) * 512],
                                          ps[:])
        vcol = 2 * HPC * HD
        for kc in range(KC):
            ps = psA.tile([128, 256], f32, tag="v", name="vps")
            for k2 in range(DT // 2):
                nc.tensor.matmul(
                    ps[:],
                    xnT[:, 2 * k2:2 * k2 + 2, kc * 128:(kc + 1) * 128],
                    wqkv_sb[:, 2 * k2:2 * k2 + 2, vcol:vcol + 256],
                    start=(k2 == 0), stop=(k2 == DT // 2 - 1),
                    perf_mode=DR)
            nc.vector.tensor_copy(vaug[:, kc, :, 0:HD], ps[:])
        psA.release()

        # prefetch ffn weights during attention
        hxp = tc.alloc_tile_pool(name="hxp", bufs=1)
        w1p = tc.alloc_tile_pool(name="w1p", bufs=1)
        w1_sb = w1p.tile([128, DT, FF], f16, tag="w1", name="w1")
        for k in range(DT):
            nc.sync.dma_start(w1_sb[:, k, :],
                              w1_in.ap()[k * 128:(k + 1) * 128, :])

        # ---------- phase D: attention (causal, descending qb) ----------
        psD = tc.alloc_tile_pool(name="psD", bufs=1, space="PSUM")
        h_sb = [hxp.tile([128, D], f16, tag=f"h{i}", name=f"h{i}")
                for i in range(4)]
        pjpart = [dp.tile([G * D, 128], f16, name=f"pjpart{i}")
                  for i in range(NQB)]
        hpart = [dp.tile([D, 128], f16, name=f"hpart{i}")
                 for i in range(NQB)]

        def attend_qb(qb):
            c2max = 2 * qb + 2
            o_sb = wrk.tile([128, 2, 512], f8, tag="o_sb", name="o_sb",
                            bufs=2)
            for h in range(HPC):
                p, hl = h // 2, (h % 2) * 64
                av = psD.tile([HD + 1, 512], f32, tag="av", name="avps",
                              bufs=1)
                pts = [None] * c2max
                for c2 in range(c2max):
                    sc = psD.tile([128, 1024], f32, tag="sc", name="scps",
                                  bufs=2)
                    for jj in range(2):
                        c = 2 * c2 + jj
                        nc.tensor.matmul(
                            sc[:, jj * 512:(jj + 1) * 512],
                            kt[p][hl:hl + 64, c * 128:(c + 1) * 128],
                            qT[p][hl:hl + 64, qb * 512:(qb + 1) * 512],
                            start=True, stop=True)
                    pt = wrk.tile([128, 2, 512], f8, tag="pt", name="pt",
                                  bufs=3)
                    nc.scalar.activation(pt[:], sc[:], AF.Exp, scale=SCALE2)
                    if c2 >= 2 * qb:
                        dk = (c2 - 2 * qb) * 2
                        nc.vector.tensor_tensor(
                            pt[:], pt[:],
                            maskd_sb[:, qb * 4 + dk:qb * 4 + dk + 2, :],
                            op=OP.mult)
                    pts[c2] = pt
                    if c2 >= 1:
                        nc.tensor.matmul(
                            av[:], vaug[:, 2 * (c2 - 1):2 * c2, h, 0:HD + 1],
                            pts[c2 - 1][:],
                            start=(c2 - 1 == 0), stop=False, perf_mode=DR)
                nc.tensor.matmul(
                    av[:], vaug[:, 2 * (c2max - 1):2 * c2max, h, 0:HD + 1],
                    pts[c2max - 1][:],
                    start=(c2max == 1), stop=True, perf_mode=DR)
                # divide by denominator (row HD), cast fp8, place into o_sb
                recip = wrk.tile([1, 512], f32, tag="recip", name="recip")
                nc.vector.reciprocal(recip[:], av[HD:HD + 1, :])
                rb = wrk.tile([64, 512], f32, tag="rb", name="rb")
                nc.gpsimd.partition_broadcast(rb[:], recip[:])
                oT = wrk.tile([64, 512], f8, tag="oT", name="oT")
                nc.vector.tensor_tensor(oT[:], av[0:HD, :], rb[:],
                                        op=OP.mult)
                nc.sync.dma_start(o_sb[hl:hl + 64, h // 2, :], oT[:])
            # proj partials for this query block -> fp16 -> reduce-scatter
            pj_sb = wrk.tile([128, DT, 4, 128], f16, tag="pj", name="pj",
                             bufs=2)
            for dc in range(DT):
                ps = psD.tile([128, 512], f32, tag="pj", name="pjps", bufs=1)
                nc.tensor.matmul(ps[:], wp_sb[:, :, dc * 128:(dc + 1) * 128],
                                 o_sb[:], start=True, stop=True, perf_mode=DR)
                nc.vector.tensor_copy(pj_sb[:, dc, :, :], ps[:])
            for s in range(G):
                nc.sync.dma_start(
                    pjpart[qb][s * D:(s + 1) * D, :].rearrange(
                        "(c p) t -> p c t", p=128),
                    pj_sb[:, :, s, :])
            nc.gpsimd.collective_compute(
                "ReduceScatter", OP.add, replica_groups=groups,
                ins=[pjpart[qb][:].opt()], outs=[hpart[qb][:].opt()])

        def finish_strip(qb):
            """h = x + proj_rs^T for own strip of block qb."""
            hp = wrk.tile([128, DT, 128], f16, tag="hp", name="hp", bufs=2)
            nc.sync.dma_start(
                hp[:], hpart[qb][:].rearrange("(c p) t -> p c t", p=128))
            for dc in range(DT):
                ps = psD.tile([128, 128], f16, tag="tr", name="trD", bufs=2)
                nc.tensor.transpose(ps[:], hp[:, dc, :], identity[:])
                nc.vector.tensor_tensor(
                    h_sb[qb][:, dc * 128:(dc + 1) * 128], ps[:],
                    x_sb[qb][:, dc * 128:(dc + 1) * 128], op=OP.add)

        for qb in (3, 2, 1, 0):
            attend_qb(qb)
            if qb == 1:
                finish_strip(3)
            elif qb == 0:
                finish_strip(2)
        finish_strip(1)
        finish_strip(0)

        xnp.release()

        # ---------- phase G: LN2 -> yT ----------
        ynp = tc.alloc_tile_pool(name="ynp", bufs=1)
        yT = ynp.tile([128, DT, T], f16, tag="yT", name="yT")
        for i in (3, 2, 1, 0):
            yn = layer_norm_tile(h_sb[i], f"ln2_{i}")
            for dc in range(DT):
                ps = psD.tile([128, 128], f16, tag="tr", name="trG", bufs=2)
                nc.tensor.transpose(ps[:], yn[:, dc * 128:(dc + 1) * 128],
                                    identity[:])
                nc.vector.tensor_copy(yT[:, dc, i * 128:(i + 1) * 128], ps[:])
        attp.release()
        wqp.release()
        shp.release()
        xpool.release()
        psD.release()

        # ---------- phase H: ffn1 + gelu (fp16) ----------
        psH = tc.alloc_tile_pool(name="psH", bufs=2, space="PSUM")
        hidp = tc.alloc_tile_pool(name="hidp", bufs=1)
        hidT = hidp.tile([128, FT, T], f16, tag="hidT", name="hidT")
        for m in range(FT):
            ps = psH.tile([128, 512], f32, tag="f1", name="f1ps")
            for k in range(DT):
                nc.tensor.matmul(ps[:],
                                 w1_sb[:, k, m * 128:(m + 1) * 128],
                                 yT[:, k, :],
                                 start=(k == 0), stop=(k == DT - 1))
            nc.scalar.activation(hidT[:, m, :], ps[:], AF.Gelu)

        # ---------- phase I: ffn2 + residual -> out ----------
        w2p = tc.alloc_tile_pool(name="w2p", bufs=2)
        for nq in range(4):
            w2_sb = w2p.tile([128, FT, 256], f16, tag="w2", name="w2")
            for m in range(FT):
                nc.sync.dma_start(
                    w2_sb[:, m, :],
                    w2_in.ap()[m * 128:(m + 1) * 128,
                               nq * 256:(nq + 1) * 256])
            for i in range(4):
                ps = psH.tile([128, 256], f32, tag="f2", name="f2ps")
                for m in range(FT):
                    nc.tensor.matmul(ps[:],
                                     hidT[:, m, i * 128:(i + 1) * 128],
                                     w2_sb[:, m, :],
                                     start=(m == 0), stop=(m == FT - 1))
                o_sb = wrk.tile([128, 256], f32, tag="fo", name="fo")
                nc.vector.tensor_tensor(
                    o_sb[:], ps[:], h_sb[i][:, nq * 256:(nq + 1) * 256],
                    op=OP.add)
                nc.sync.dma_start(
                    out_t.ap()[i * 128:(i + 1) * 128,
                               nq * 256:(nq + 1) * 256], o_sb[:])
        psH.release()
        w2p.release()
        hidp.release()
        ynp.release()
        w1p.release()
        hxp.release()
        dp.release()
        wrk.release()
        pp.release()

    nc.compile()
    return nc


def _prep_fast(inputs):
    import ml_dtypes
    f8 = ml_dtypes.float8_e4m3

    x = np.asarray(inputs["x"], np.float32)
    mask = np.asarray(inputs["mask"])
    qkv_w = np.asarray(inputs["qkv_w"], np.float32)
    proj_w = np.asarray(inputs["proj_w"], np.float32)
    w1_16 = np.asarray(inputs["ffn_w1"], np.float32).astype(np.float16)
    w2_16 = np.asarray(inputs["ffn_w2"], np.float32).astype(np.float16)

    def to8(a):
        return np.ascontiguousarray(np.clip(a, -240, 240)).astype(f8)

    in_maps = []
    for r in range(N_CORES):
        b, j = r // G, r % G
        rows = np.concatenate(
            [np.arange(qb * 512 + j * 128, qb * 512 + j * 128 + 128)
             for qb in range(NQB)])
        wq = qkv_w[:, 256 * j: 256 * j + 256]
        wk = qkv_w[:, D + 256 * j: D + 256 * j + 256]
        wv = qkv_w[:, 2 * D + 256 * j: 2 * D + 256 * j + 256]
        maskd = np.concatenate(
            [(mask[b, 0, qb * 512:(qb + 1) * 512,
                   qb * 512:(qb + 1) * 512] != 0).T.astype(np.float32)
             for qb in range(NQB)], axis=0)
        im = {
            "x": np.ascontiguousarray(x[b, rows, :]),
            "maskd": maskd.astype(f8),
            "wqkv": to8(np.concatenate([wq, wk, wv], axis=1)),
            "wp": to8(proj_w[256 * j: 256 * j + 256, :]),
            "w_f1": w1_16, "w_f2": w2_16,
        }
        in_maps.append(im)
    return in_maps


def _gather_fast(res):
    out = np.empty((B, L, D), np.float32)
    for r in range(N_CORES):
        b, j = r // G, r % G
        o = res.results[r]["out"]
        for qb in range(NQB):
            out[b, qb * 512 + j * 128: qb * 512 + j * 128 + 128, :] = \
                o[qb * 128:(qb + 1) * 128, :]
    return out


def _fast_ok(inputs):
    """Fast path requires exact causal mask + trivial biases/affine."""
    mask = np.asarray(inputs["mask"])
    if mask.shape != (B, 1, L, L):
        return False
    tril = np.tril(np.ones((L, L), mask.dtype))
    for b in range(B):
        if not np.array_equal(mask[b, 0], tril):
            return False
    return (np.allclose(np.asarray(inputs["ln1_w"]), 1.0)
            and not np.any(np.asarray(inputs["ln1_b"]))
            and np.allclose(np.asarray(inputs["ln2_w"]), 1.0)
            and not np.any(np.asarray(inputs["ln2_b"]))
            and not np.any(np.asarray(inputs["qkv_b"]))
            and not np.any(np.asarray(inputs["proj_b"]))
            and not np.any(np.asarray(inputs["ffn_b1"]))
            and not np.any(np.asarray(inputs["ffn_b2"])))


# ------------------------------------------------------------------
# generic fallback: token-parallel kernel (previous version, verbatim)
# ------------------------------------------------------------------
def _build(cfg):
    import concourse.bass as bass
    from concourse import bacc, mybir
    import concourse.tile as tile
    from concourse.masks import make_identity

    f32 = mybir.dt.float32
    f16 = mybir.dt.float16
    AF = mybir.ActivationFunctionType
    OP = mybir.AluOpType

    TT = T // 128            # 4 token tiles
    DT = D // 128            # 8 dim chunks
    QKF = 2 * D              # q+k features
    KCg = L // 128           # 16 key chunks
    FT = FF // 128           # 32 ffn hidden chunks

    nc = bacc.Bacc("TRN2", target_bir_lowering=False, debug=False,
                   num_devices=N_CORES)

    x_in = nc.dram_tensor("x", [T, D], f32, kind="ExternalInput")
    mask_in = nc.dram_tensor("maskT", [L, T], f16, kind="ExternalInput")
    wqk_in = nc.dram_tensor("w_qk", [D, QKF], f16, kind="ExternalInput")
    wv_in = nc.dram_tensor("w_v", [D, D], f16, kind="ExternalInput")
    wp_in = nc.dram_tensor("w_proj", [D, D], f16, kind="ExternalInput")
    w1_in = nc.dram_tensor("w_f1", [D, FF], f16, kind="ExternalInput")
    w2_in = nc.dram_tensor("w_f2", [FF, D], f16, kind="ExternalInput")
    out_t = nc.dram_tensor("out", [T, D], f32, kind="ExternalOutput")

    opt = {}
    if cfg["ln1_affine"]:
        opt["ln1_wb"] = nc.dram_tensor("ln1_wb", [2, D], f32, kind="ExternalInput")
    if cfg["ln2_affine"]:
        opt["ln2_wb"] = nc.dram_tensor("ln2_wb", [2, D], f32, kind="ExternalInput")
    if cfg["qkv_bias"]:
        opt["bqk"] = nc.dram_tensor("bqk", [QKF, 1], f32, kind="ExternalInput")
        opt["bv"] = nc.dram_tensor("bv", [1, D], f32, kind="ExternalInput")
    if cfg["proj_bias"]:
        opt["bproj"] = nc.dram_tensor("bproj", [1, D], f32, kind="ExternalInput")
    if cfg["ffn1_bias"]:
        opt["bf1"] = nc.dram_tensor("bf1", [FF, 1], f32, kind="ExternalInput")
    if cfg["ffn2_bias"]:
        opt["bf2"] = nc.dram_tensor("bf2", [1, D], f32, kind="ExternalInput")

    with tile.TileContext(nc) as tc:
        pp = tc.alloc_tile_pool(name="persist", bufs=1)
        wp = tc.alloc_tile_pool(name="work", bufs=3)
        wtp = tc.alloc_tile_pool(name="wtile", bufs=6)
        pszoo = tc.alloc_tile_pool(name="psums", bufs=2, space="PSUM")
        dp = tc.alloc_tile_pool(name="dram", bufs=1, space="DRAM")

        identity = pp.tile([128, 128], f16, tag="identity", name="identity")
        make_identity(nc, identity[:])
        eps_sb = pp.tile([128, 1], f32, tag="eps", name="eps")
        nc.vector.memset(eps_sb[:], EPS)

        def bcast_tile(src_ap, n, tag):
            row = pp.tile([1, n], f32, tag=tag + "r", name=tag + "r")
            nc.sync.dma_start(row[:], src_ap)
            t_ = pp.tile([128, n], f32, tag=tag, name=tag)
            nc.gpsimd.partition_broadcast(t_[:], row[:])
            return t_

        ln1_w_bc = ln1_b_bc = ln2_w_bc = ln2_b_bc = None
        if cfg["ln1_affine"]:
            ln1_w_bc = bcast_tile(opt["ln1_wb"].ap()[0:1, :], D, "ln1w")
            ln1_b_bc = bcast_tile(opt["ln1_wb"].ap()[1:2, :], D, "ln1b")
        if cfg["ln2_affine"]:
            ln2_w_bc = bcast_tile(opt["ln2_wb"].ap()[0:1, :], D, "ln2w")
            ln2_b_bc = bcast_tile(opt["ln2_wb"].ap()[1:2, :], D, "ln2b")
        bv_bc = bcast_tile(opt["bv"].ap(), D, "bv") if cfg["qkv_bias"] else None
        bp_bc = bcast_tile(opt["bproj"].ap(), D, "bp") if cfg["proj_bias"] else None
        bf2_bc = bcast_tile(opt["bf2"].ap(), D, "bf2") if cfg["ffn2_bias"] else None
        bqk_sb = None
        if cfg["qkv_bias"]:
            bqk_sb = [pp.tile([128, 1], f32, tag=f"bqk{f}", name=f"bqk{f}")
                      for f in range(QKF // 128)]
            for f in range(QKF // 128):
                nc.sync.dma_start(bqk_sb[f][:],
                                  opt["bqk"].ap()[f * 128:(f + 1) * 128, :])
        bf1_sb = None
        if cfg["ffn1_bias"]:
            bf1_sb = [pp.tile([128, 1], f32, tag=f"bf1{m}", name=f"bf1{m}")
                      for m in range(FT)]
            for m in range(FT):
                nc.sync.dma_start(bf1_sb[m][:],
                                  opt["bf1"].ap()[m * 128:(m + 1) * 128, :])

        def layer_norm_T(src_tiles, dstT_tiles, w_bc, b_bc, affine, tag):
            for i in range(TT):
                xt = src_tiles[i]
                mu = wp.tile([128, 1], f32, tag="lnmu", name="lnmu")
                nc.vector.tensor_reduce(mu[:], xt[:], mybir.AxisListType.X, OP.add)
                nc.vector.tensor_scalar_mul(mu[:], mu[:], 1.0 / D)
                junk = wp.tile([128, D], f16, tag="lnjunk", name="lnjunk", bufs=1)
                varr = wp.tile([128, 1], f32, tag="lnvar", name="lnvar")
                nc.vector.scalar_tensor_tensor(
                    junk[:], xt[:], mu[:], xt[:],
                    op0=OP.subtract, op1=OP.mult, accum_out=varr[:])
                std = wp.tile([128, 1], f32, tag="lnstd", name="lnstd")
                nc.scalar.activation(std[:], varr[:], AF.Sqrt,
                                     bias=eps_sb[:], scale=1.0 / D)
                rstd = wp.tile([128, 1], f32, tag="lnrstd", name="lnrstd")
                nc.vector.reciprocal(rstd[:], std[:])
                xn = wp.tile([128, D], f16, tag="lnxn", name="lnxn")
                nc.vector.tensor_scalar(xn[:], xt[:], mu[:], rstd[:],
                                        op0=OP.subtract, op1=OP.mult)
                if affine:
                    nc.vector.tensor_tensor(xn[:], xn[:], w_bc[:], op=OP.mult)
                    nc.vector.tensor_tensor(xn[:], xn[:], b_bc[:], op=OP.add)
                for j in range(DT):
                    ps = pszoo.tile([128, 128], f16, tag="tr", name="tr")
                    nc.tensor.transpose(ps[:], xn[:, j * 128:(j + 1) * 128],
                                        identity[:])
                    nc.vector.tensor_copy(
                        dstT_tiles[j][:, i * 128:(i + 1) * 128], ps[:])

        qkv_pool = tc.alloc_tile_pool(name="qkvp", bufs=1)
        xnT_pool = tc.alloc_tile_pool(name="xnT", bufs=1)
        wv_pool = tc.alloc_tile_pool(name="wvp", bufs=1)
        x_sb = [pp.tile([128, D], f32, tag=f"x{i}", name=f"x{i}")
                for i in range(TT)]
        for i in range(TT):
            nc.sync.dma_start(x_sb[i][:], x_in.ap()[i * 128:(i + 1) * 128, :])
        xnT = [xnT_pool.tile([128, T], f16, tag=f"xnT{j}", name=f"xnT{j}")
               for j in range(DT)]
        layer_norm_T(x_sb, xnT, ln1_w_bc, ln1_b_bc, cfg["ln1_affine"], "ln1")

        qkT = [qkv_pool.tile([128, T], f16, tag=f"qkT{f}", name=f"qkT{f}")
               for f in range(QKF // 128)]
        for f in range(QKF // 128):
            ps = pszoo.tile([128, T], f32, tag="acc", name="accqk")
            for j in range(DT):
                wt = wtp.tile([128, 128], f16, tag="wqk", name="wqk")
                nc.sync.dma_start(
                    wt[:], wqk_in.ap()[j * 128:(j + 1) * 128,
                                       f * 128:(f + 1) * 128])
                nc.tensor.matmul(ps[:], wt[:], xnT[j][:],
                                 start=(j == 0), stop=(j == DT - 1))
            if cfg["qkv_bias"]:
                nc.vector.tensor_scalar_add(qkT[f][:], ps[:], bqk_sb[f][:])
            else:
                nc.vector.tensor_copy(qkT[f][:], ps[:])

        qT_sb = [qkv_pool.tile([64, T], f16, tag=f"qT{h}", name=f"qT{h}")
                 for h in range(H)]
        for h in range(H):
            lo = (h % 2) * 64
            nc.sync.dma_start(qT_sb[h][:], qkT[h // 2][lo:lo + 64, :])

        wv_sb = [wv_pool.tile([128, 512], f16, tag=f"wv{k}", name=f"wv{k}")
                 for k in range(DT * 2)]
        for j in range(DT):
            for n in range(2):
                nc.sync.dma_start(
                    wv_sb[j * 2 + n][:],
                    wv_in.ap()[j * 128:(j + 1) * 128, n * 512:(n + 1) * 512])
        v_sb = [qkv_pool.tile([128, D], f16, tag=f"v{i}", name=f"v{i}")
                for i in range(TT)]
        for i in range(TT):
            for n in range(2):
                ps = pszoo.tile([128, 512], f32, tag="acc", name="accv")
                for j in range(DT):
                    nc.tensor.matmul(ps[:], xnT[j][:, i * 128:(i + 1) * 128],
                                     wv_sb[j * 2 + n][:],
                                     start=(j == 0), stop=(j == DT - 1))
                dst = v_sb[i][:, n * 512:(n + 1) * 512]
                if cfg["qkv_bias"]:
                    nc.vector.tensor_tensor(dst, ps[:],
                                            bv_bc[:, n * 512:(n + 1) * 512],
                                            op=OP.add)
                else:
                    nc.vector.tensor_copy(dst, ps[:])

        groups = [list(range(G)), list(range(G, 2 * G))]
        kt_shard = dp.tile([D, T], f16)
        v_shard = dp.tile([T, D], f16)
        for f in range(DT):
            nc.sync.dma_start(kt_shard[f * 128:(f + 1) * 128, :], qkT[DT + f][:])
        for i in range(TT):
            nc.sync.dma_start(v_shard[i * 128:(i + 1) * 128, :], v_sb[i][:])
        kt_g = dp.tile([G * D, T], f16)
        v_g = dp.tile([G * T, D], f16)
        nc.gpsimd.collective_compute(
            "AllGather", OP.bypass, replica_groups=groups,
            ins=[kt_shard[:].opt()], outs=[kt_g[:].opt()])
        nc.gpsimd.collective_compute(
            "AllGather", OP.bypass, replica_groups=groups,
            ins=[v_shard[:].opt()], outs=[v_g[:].opt()])
        wv_pool.release()
        xnT_pool.release()

        attn_pool = tc.alloc_tile_pool(name="attnp", bufs=1)
        mask_pool = tc.alloc_tile_pool(name="maskp", bufs=1)
        mask_sb = [mask_pool.tile([128, T], f16, tag=f"m{c}", name=f"m{c}")
                   for c in range(KCg)]
        for c in range(KCg):
            nc.sync.dma_start(mask_sb[c][:],
                              mask_in.ap()[c * 128:(c + 1) * 128, :])

        attnT = [attn_pool.tile([128, T], f16, tag=f"aT{j}", name=f"aT{j}")
                 for j in range(DT)]
        v_g_r = v_g[:].rearrange("(c p) n -> p c n", p=128)
        for h in range(H):
            kt_h = wp.tile([64, L], f16, tag="kt_h", name="kt_h", bufs=2)
            for g in range(G):
                nc.sync.dma_start(
                    kt_h[:, g * T:(g + 1) * T],
                    kt_g[g * D + h * HD:g * D + (h + 1) * HD, :])
            vaug = wp.tile([128, KCg, HD + 1], f16, tag="vaug", name="vaug",
                           bufs=2)
            nc.vector.memset(vaug[:, :, HD:HD + 1], 1.0)
            nc.sync.dma_start(vaug[:, :, 0:HD],
                              v_g_r[:, :, h * HD:(h + 1) * HD])

            o_ps = pszoo.tile([HD + 1, T], f32, tag="ops", name="ops")
            for c in range(KCg):
                s_ps = pszoo.tile([128, T], f32, tag="sps", name="sps")
                nc.tensor.matmul(s_ps[:], kt_h[:, c * 128:(c + 1) * 128],
                                 qT_sb[h][:], start=True, stop=True)
                pt = wp.tile([128, T], f16, tag="pt", name="pt")
                nc.scalar.activation(pt[:], s_ps[:], AF.Exp, scale=SCALE2)
                nc.vector.tensor_tensor(pt[:], pt[:], mask_sb[c][:], op=OP.mult)
                nc.tensor.matmul(o_ps[:], vaug[:, c:c + 1, :], pt[:],
                                 start=(c == 0), stop=(c == KCg - 1))
            recip = wp.tile([1, T], f32, tag="recip", name="recip", bufs=2)
            nc.vector.reciprocal(recip[:], o_ps[HD:HD + 1, :])
            rb = wp.tile([64, T], f32, tag="rb", name="rb", bufs=2)
            nc.gpsimd.partition_broadcast(rb[:], recip[:])
            oT_h = wp.tile([64, T], f16, tag="oT_h", name="oT_h", bufs=2)
            nc.vector.tensor_tensor(oT_h[:], o_ps[0:HD, :], rb[:], op=OP.mult)
            lo = (h % 2) * 64
            nc.sync.dma_start(attnT[h // 2][lo:lo + 64, :], oT_h[:])
        mask_pool.release()

        wpp = tc.alloc_tile_pool(name="wpp", bufs=1)
        wproj_sb = [wpp.tile([128, 512], f16, tag=f"wp{k}", name=f"wp{k}")
                    for k in range(DT * 2)]
        for j in range(DT):
            for n in range(2):
                nc.sync.dma_start(
                    wproj_sb[j * 2 + n][:],
                    wp_in.ap()[j * 128:(j + 1) * 128, n * 512:(n + 1) * 512])
        h_sb = [pp.tile([128, D], f32, tag=f"h{i}", name=f"h{i}")
                for i in range(TT)]
        for i in range(TT):
            for n in range(2):
                ps = pszoo.tile([128, 512], f32, tag="acc", name="accp")
                for j in range(DT):
                    nc.tensor.matmul(ps[:], attnT[j][:, i * 128:(i + 1) * 128],
                                     wproj_sb[j * 2 + n][:],
                                     start=(j == 0), stop=(j == DT - 1))
                dst = h_sb[i][:, n * 512:(n + 1) * 512]
                xsl = x_sb[i][:, n * 512:(n + 1) * 512]
                if cfg["proj_bias"]:
                    nc.vector.tensor_tensor(dst, ps[:],
                                            bp_bc[:, n * 512:(n + 1) * 512],
                                            op=OP.add)
                    nc.vector.tensor_tensor(dst, dst, xsl, op=OP.add)
                else:
                    nc.vector.tensor_tensor(dst, ps[:], xsl, op=OP.add)
        wpp.release()
        attn_pool.release()
        qkv_pool.release()

        hid_pool = tc.alloc_tile_pool(name="hidp", bufs=1)
        yT_pool = tc.alloc_tile_pool(name="yTp", bufs=1)
        yT = [yT_pool.tile([128, T], f16, tag=f"yT{j}", name=f"yT{j}")
              for j in range(DT)]
        layer_norm_T(h_sb, yT, ln2_w_bc, ln2_b_bc, cfg["ln2_affine"], "ln2")

        hidT = [hid_pool.tile([128, T], f16, tag=f"hidT{m}", name=f"hidT{m}")
                for m in range(FT)]
        for m in range(FT):
            ps = pszoo.tile([128, T], f32, tag="acc", name="accf1")
            for j in range(DT):
                wt = wtp.tile([128, 128], f16, tag="w1", name="w1")
                nc.sync.dma_start(
                    wt[:], w1_in.ap()[j * 128:(j + 1) * 128,
                                      m * 128:(m + 1) * 128])
                nc.tensor.matmul(ps[:], wt[:], yT[j][:],
                                 start=(j == 0), stop=(j == DT - 1))
            if cfg["ffn1_bias"]:
                nc.scalar.activation(hidT[m][:], ps[:], AF.Gelu,
                                     bias=bf1_sb[m][:])
            else:
                nc.scalar.activation(hidT[m][:], ps[:], AF.Gelu)
        yT_pool.release()

        w2_pool = tc.alloc_tile_pool(name="w2p", bufs=1)
        for n in range(2):
            w2_sb = [w2_pool.tile([128, 512], f16, tag=f"w2_{m}",
                                  name=f"w2_{m}") for m in range(FT)]
            for m in range(FT):
                nc.sync.dma_start(
                    w2_sb[m][:], w2_in.ap()[m * 128:(m + 1) * 128,
                                            n * 512:(n + 1) * 512])
            for i in range(TT):
                ps = pszoo.tile([128, 512], f32, tag="acc", name="accf2")
                for m in range(FT):
                    nc.tensor.matmul(ps[:], hidT[m][:, i * 128:(i + 1) * 128],
                                     w2_sb[m][:],
                                     start=(m == 0), stop=(m == FT - 1))
                o_sb = wp.tile([128, 512], f32, tag="o_sb", name="o_sb")
                hsl = h_sb[i][:, n * 512:(n + 1) * 512]
                if cfg["ffn2_bias"]:
                    nc.vector.tensor_tensor(o_sb[:], ps[:],
                                            bf2_bc[:, n * 512:(n + 1) * 512],
                                            op=OP.add)
                    nc.vector.tensor_tensor(o_sb[:], o_sb[:], hsl, op=OP.add)
                else:
                    nc.vector.tensor_tensor(o_sb[:], ps[:], hsl, op=OP.add)
                nc.sync.dma_start(
                    out_t.ap()[i * 128:(i + 1) * 128,
                               n * 512:(n + 1) * 512], o_sb[:])
        w2_pool.release()
        hid_pool.release()
        wtp.release()
        wp.release()
        pszoo.release()
        dp.release()
        pp.release()

    nc.compile()
    return nc


def _prep(inputs):
    x = np.asarray(inputs["x"], np.float32)
    mask = np.asarray(inputs["mask"])
    qkv_w = np.asarray(inputs["qkv_w"], np.float32)
    qkv_b = np.asarray(inputs["qkv_b"], np.float32)
    proj_w = np.asarray(inputs["proj_w"], np.float32)
    proj_b = np.asarray(inputs["proj_b"], np.float32)
    ffn_w1 = np.asarray(inputs["ffn_w1"], np.float32)
    ffn_b1 = np.asarray(inputs["ffn_b1"], np.float32)
    ffn_w2 = np.asarray(inputs["ffn_w2"], np.float32)
    ffn_b2 = np.asarray(inputs["ffn_b2"], np.float32)
    ln1_w = np.asarray(inputs["ln1_w"], np.float32)
    ln1_b = np.asarray(inputs["ln1_b"], np.float32)
    ln2_w = np.asarray(inputs["ln2_w"], np.float32)
    ln2_b = np.asarray(inputs["ln2_b"], np.float32)

    cfg = {
        "ln1_affine": not (np.allclose(ln1_w, 1.0) and np.allclose(ln1_b, 0.0)),
        "ln2_affine": not (np.allclose(ln2_w, 1.0) and np.allclose(ln2_b, 0.0)),
        "qkv_bias": bool(np.any(qkv_b)),
        "proj_bias": bool(np.any(proj_b)),
        "ffn1_bias": bool(np.any(ffn_b1)),
        "ffn2_bias": bool(np.any(ffn_b2)),
    }

    w_qk = np.ascontiguousarray(qkv_w[:, :2 * D]).astype(np.float16)
    w_v = np.ascontiguousarray(qkv_w[:, 2 * D:]).astype(np.float16)
    w_p16 = proj_w.astype(np.float16)
    w1_16 = ffn_w1.astype(np.float16)
    w2_16 = ffn_w2.astype(np.float16)

    in_maps = []
    for r in range(N_CORES):
        b = r // G
        row0 = (r % G) * T
        im = {
            "x": np.ascontiguousarray(x[b, row0:row0 + T, :]),
            "maskT": np.ascontiguousarray(
                (mask[b, 0, row0:row0 + T, :] != 0).T.astype(np.float16)),
            "w_qk": w_qk, "w_v": w_v, "w_proj": w_p16,
            "w_f1": w1_16, "w_f2": w2_16,
        }
        if cfg["ln1_affine"]:
            im["ln1_wb"] = np.ascontiguousarray(np.stack([ln1_w, ln1_b]))
        if cfg["ln2_affine"]:
            im["ln2_wb"] = np.ascontiguousarray(np.stack([ln2_w, ln2_b]))
        if cfg["qkv_bias"]:
            im["bqk"] = np.ascontiguousarray(qkv_b[:2 * D, None])
            im["bv"] = np.ascontiguousarray(qkv_b[None, 2 * D:])
        if cfg["proj_bias"]:
            im["bproj"] = np.ascontiguousarray(proj_b[None, :])
        if cfg["ffn1_bias"]:
            im["bf1"] = np.ascontiguousarray(ffn_b1[:, None])
        if cfg["ffn2_bias"]:
            im["bf2"] = np.ascontiguousarray(ffn_b2[None, :])
        in_maps.append(im)
    return cfg, in_maps


def _run(inputs, trace=False):
    from concourse.bass_utils import run_bass_kernel_spmd

    if _fast_ok(inputs):
        if "fast" not in _cache:
            _cache["fast"] = _build_fast()
        nc = _cache["fast"]
        in_maps = _prep_fast(inputs)
        res = run_bass_kernel_spmd(nc, in_maps, core_ids=list(range(N_CORES)),
                                   trace=trace)
        return _gather_fast(res), res

    cfg, in_maps = _prep(inputs)
    key = tuple(sorted(cfg.items()))
    if key not in _cache:
        _cache[key] = _build(cfg)
    nc = _cache[key]
    res = run_bass_kernel_spmd(nc, in_maps, core_ids=list(range(N_CORES)),
                               trace=trace)
    out = np.empty((B, L, D), np.float32)
    for r in range(N_CORES):
        b = r // G
        row0 = (r % G) * T
        out[b, row0:row0 + T, :] = res.results[r]["out"]
    return out, res


def kernel(**inputs):
    out, _ = _run(inputs, trace=False)
    return out


# revision 24
# speedup vs baseline: 1.7972x; 1.1238x over previous
"""Distributed Trainium2 kernel for a dense transformer block.

Problem: x[2,2048,1024] -> LN1 -> MHA(16 heads, causal) -> +res -> LN2 ->
FFN(4096, gelu) -> +res.

Fast path (mask verified causal-tril on host, biases zero, LN affine
identity): head-parallel attention. Core r (batch b=r//4, rank j=r%4) owns
heads 4j..4j+3 for all 2048 queries of its batch, and token strips
{qb*512 + j*128 .. +128} for the token-parallel parts (LN/FFN/residual).

- LN1 runs sequence-parallel on own 512 tokens; xn^T is all-gathered in fp8
  (one small collective instead of two fp16 K/V gathers).
- QKV/proj gemms run in fp8 with DoubleRow (2 contraction rows per
  partition); scores in plain fp8 (K=64); AV in fp8 DoubleRow with a ones
  column appended to V so the softmax denominator accumulates for free.
- Causality is exploited uniformly across cores: query block qb only
  touches key chunks 0..4qb+3, so scores/exp/AV work halves. Only the
  diagonal 512x512 block is masked (with real mask data).
- proj partials are reduce-scattered per query block (4 pipelined fp16
  collectives, descending qb so the last one is the cheapest block).
- FFN stays fp16 (fp8 fails the tolerance there), with wide weight DMAs.

Generic fallback (non-causal mask or non-trivial biases): the previous
token-parallel kernel, kept verbatim below.
"""

import sys

sys.path.insert(0, "/opt/trn_rl_repo")

import numpy as np

B, L, D = 2, 2048, 1024
H, HD = 16, 64
FF = 4 * D
N_CORES = 8
G = N_CORES // B                # 4 cores per batch group
T = (B * L) // N_CORES          # 512 tokens per core
HPC = H // G                    # 4 heads per core
NQB = L // 512                  # 4 query blocks
KC = L // 128                   # 16 key chunks
SCALE2 = float(HD) ** -0.5
EPS = 1e-5

_cache = {}


# ------------------------------------------------------------------
# fast path: head-parallel causal kernel
# ------------------------------------------------------------------
def _build_fast():
    import concourse.bass as bass
    from concourse import bacc, mybir
    import concourse.tile as tile
    from concourse.masks import make_identity

    f32 = mybir.dt.float32
    f16 = mybir.dt.float16
    f8 = mybir.dt.float8e4
    AF = mybir.ActivationFunctionType
    OP = mybir.AluOpType
    DR = mybir.MatmulPerfMode.DoubleRow

    DT = D // 128               # 8 dim chunks
    FT = FF // 128              # 32 ffn hidden chunks

    nc = bacc.Bacc("TRN2", target_bir_lowering=False, debug=False,
                   num_devices=N_CORES)

    x_in = nc.dram_tensor("x", [T, D], f32, kind="ExternalInput")
    maskd_in = nc.dram_tensor("maskd", [NQB * 512, 512], f16,
                              kind="ExternalInput")
    wqkv_in = nc.dram_tensor("wqkv", [D, 3 * HPC * HD], f8,
                             kind="ExternalInput")
    wp_in = nc.dram_tensor("wp", [HPC * HD, D], f16, kind="ExternalInput")
    w1_in = nc.dram_tensor("w_f1", [D, FF], f16, kind="ExternalInput")
    w2_in = nc.dram_tensor("w_f2", [FF, D], f16, kind="ExternalInput")
    out_t = nc.dram_tensor("out", [T, D], f32, kind="ExternalOutput")

    groups = [list(range(G)), list(range(G, 2 * G))]

    with tile.TileContext(nc) as tc:
        pp = tc.alloc_tile_pool(name="persist", bufs=1)
        wrk = tc.alloc_tile_pool(name="work", bufs=2)
        dp = tc.alloc_tile_pool(name="dram", bufs=1, space="DRAM")

        identity = pp.tile([128, 128], f16, tag="identity", name="identity")
        make_identity(nc, identity[:])
        eps_sb = pp.tile([128, 1], f32, tag="eps", name="eps")
        nc.vector.memset(eps_sb[:], EPS)
        ones_sb = pp.tile([128, HD], f16, tag="ones", name="ones")
        nc.vector.memset(ones_sb[:], 1.0)

        # ---------- phase A: x load + LN1 + transpose to fp8 shard ----------
        psA = tc.alloc_tile_pool(name="psA", bufs=2, space="PSUM")
        xpool = tc.alloc_tile_pool(name="xp", bufs=1, side="right")
        shp = tc.alloc_tile_pool(name="shp", bufs=1, side="right")

        x_sb = [xpool.tile([128, D], f32, tag=f"x{i}", name=f"x{i}")
                for i in range(4)]
        for i in range(4):
            nc.sync.dma_start(x_sb[i][:], x_in.ap()[i * 128:(i + 1) * 128, :])

        def layer_norm_tile(xt, tag):
            """xt [128, D] f32 -> normalized [128, D] f16."""
            mu = wrk.tile([128, 1], f32, tag="lnmu", name="lnmu")
            nc.vector.tensor_reduce(mu[:], xt[:], mybir.AxisListType.X, OP.add)
            nc.vector.tensor_scalar_mul(mu[:], mu[:], 1.0 / D)
            junk = wrk.tile([128, D], f16, tag="lnjunk", name="lnjunk", bufs=1)
            varr = wrk.tile([128, 1], f32, tag="lnvar", name="lnvar")
            nc.vector.scalar_tensor_tensor(
                junk[:], xt[:], mu[:], xt[:],
                op0=OP.subtract, op1=OP.mult, accum_out=varr[:])
            std = wrk.tile([128, 1], f32, tag="lnstd", name="lnstd")
            nc.scalar.activation(std[:], varr[:], AF.Sqrt,
                                 bias=eps_sb[:], scale=1.0 / D)
            rstd = wrk.tile([128, 1], f32, tag="lnrstd", name="lnrstd")
            nc.vector.reciprocal(rstd[:], std[:])
            xn = wrk.tile([128, D], f16, tag="lnxn", name="lnxn")
            nc.vector.tensor_scalar(xn[:], xt[:], mu[:], rstd[:],
                                    op0=OP.subtract, op1=OP.mult)
            return xn

        xsh_sb = shp.tile([128, DT, T], f8, tag="xsh", name="xsh")
        for i in range(4):
            xn = layer_norm_tile(x_sb[i], f"ln1_{i}")
            for dc in range(DT):
                ps = psA.tile([128, 128], f16, tag="tr", name="trA")
                nc.tensor.transpose(ps[:], xn[:, dc * 128:(dc + 1) * 128],
                                    identity[:])
                nc.vector.tensor_copy(
                    xsh_sb[:, dc, i * 128:(i + 1) * 128], ps[:])

        xshard = dp.tile([D, T], f8)
        nc.sync.dma_start(
            xshard[:].rearrange("(c p) t -> p c t", p=128), xsh_sb[:])
        xg = dp.tile([G * D, T], f8)
        nc.gpsimd.collective_compute(
            "AllGather", OP.bypass, replica_groups=groups,
            ins=[xshard[:].opt()], outs=[xg[:].opt()])

        # weights for qkv/proj/mask arrive during LN1/AG
        wqp = tc.alloc_tile_pool(name="wqp", bufs=1, side="right")
        wqkv_sb = wqp.tile([128, DT, 3 * HPC * HD], f8, tag="wqkv",
                           name="wqkv")
        for k in range(DT):
            nc.sync.dma_start(wqkv_sb[:, k, :],
                              wqkv_in.ap()[k * 128:(k + 1) * 128, :])
        attp = tc.alloc_tile_pool(name="attp", bufs=1, side="right")
        wp_sb = attp.tile([128, 2, D], f16, tag="wp", name="wp")
        for k in range(2):
            nc.sync.dma_start(wp_sb[:, k, :],
                              wp_in.ap()[k * 128:(k + 1) * 128, :])
        maskd_sb = attp.tile([128, 4 * NQB, 512], f16, tag="maskd",
                             name="maskd")
        nc.sync.dma_start(
            maskd_sb[:], maskd_in.ap().rearrange("(g p) q -> p g q", p=128))

        # gathered xn^T -> [128, DT, L]
        xnp = tc.alloc_tile_pool(name="xnp", bufs=1, side="right")
        xnT = xnp.tile([128, DT, L], f8, tag="xnT", name="xnT")
        for g in range(G):
            nc.sync.dma_start(
                xnT[:, :, g * T:(g + 1) * T],
                xg[g * D:(g + 1) * D, :].rearrange("(c p) t -> p c t", p=128))

        # ---------- phase C: QKV gemms (fp8 DoubleRow) ----------
        qT = [attp.tile([128, L], f8, tag=f"qT{p}", name=f"qT{p}")
              for p in range(2)]
        kt = [attp.tile([128, L], f8, tag=f"kt{p}", name=f"kt{p}")
              for p in range(2)]
        # vaug[:, kc, h, 0:64] = V[kc*128+p, h*64+v]; [..., 64] = 1.0
        VP = 68
        vaug = attp.tile([128, KC, HPC, VP], f16, tag="vaug", name="vaug")
        for kc in range(KC):
            nc.vector.memset(vaug[:, kc, :, HD:HD + 1], 1.0)

        for p in range(2):
            for blk in range(NQB):
                for which, dst in ((0, qT[p]), (1, kt[p])):
                    col0 = which * HPC * HD + p * 128
                    ps = psA.tile([128, 512], f32, tag="qk", name="qkps")
                    for k2 in range(DT // 2):
                        nc.tensor.matmul(
                            ps[:],
                            wqkv_sb[:, 2 * k2:2 * k2 + 2, col0:col0 + 128],
                            xnT[:, 2 * k2:2 * k2 + 2,
                                blk * 512:(blk + 1) * 512],
                            start=(k2 == 0), stop=(k2 == DT // 2 - 1),
                            perf_mode=DR)
                    nc.vector.tensor_copy(dst[:, blk * 512:(blk + 1) * 512],
                                          ps[:])
        vcol = 2 * HPC * HD
        for kc in range(KC):
            ps = psA.tile([128, 256], f32, tag="v", name="vps")
            for k2 in range(DT // 2):
                nc.tensor.matmul(
                    ps[:],
                    xnT[:, 2 * k2:2 * k2 + 2, kc * 128:(kc + 1) * 128],
                    wqkv_sb[:, 2 * k2:2 * k2 + 2, vcol:vcol + 256],
                    start=(k2 == 0), stop=(k2 == DT // 2 - 1),
                    perf_mode=DR)
            nc.vector.tensor_copy(vaug[:, kc, :, 0:HD], ps[:])
        psA.release()

        # prefetch ffn weights during attention
        hxp = tc.alloc_tile_pool(name="hxp", bufs=1)
        w1p = tc.alloc_tile_pool(name="w1p", bufs=1)
        w1_sb = w1p.tile([128, DT, FF], f16, tag="w1", name="w1")
        for k in range(DT):
            nc.sync.dma_start(w1_sb[:, k, :],
                              w1_in.ap()[k * 128:(k + 1) * 128, :])

        # ---------- phase D: attention (causal, descending qb) ----------
        psD = tc.alloc_tile_pool(name="psD", bufs=1, space="PSUM")
        h_sb = [hxp.tile([128, D], f16, tag=f"h{i}", name=f"h{i}")
                for i in range(4)]
        pjpart = [dp.tile([G * D, 128], f16, name=f"pjpart{i}")
                  for i in range(NQB)]
        hpart = [dp.tile([D, 128], f16, name=f"hpart{i}")
                 for i in range(NQB)]

        o_sbs = {}

        def attend_heads(qb):
            c2max = 2 * qb + 2
            o_sb = wrk.tile([128, 2, 512], f16, tag="o_sb", name="o_sb",
                            bufs=2)
            o_sbs[qb] = o_sb
            den = wrk.tile([128, 512], f32, tag="den", name="den", bufs=2)
            nc.vector.memset(den[:], 1.0)
            oraw = [None] * HPC
            for h in range(HPC):
                p, hl = h // 2, (h % 2) * 64
                av = psD.tile([HD + 1, 512], f32, tag="av", name="avps",
                              bufs=2)
                pts = [None] * c2max
                for c2 in range(c2max):
                    sc = psD.tile([128, 1024], f32, tag="sc", name="scps",
                                  bufs=2)
                    for jj in range(2):
                        c = 2 * c2 + jj
                        nc.tensor.matmul(
                            sc[:, jj * 512:(jj + 1) * 512],
                            kt[p][hl:hl + 64, c * 128:(c + 1) * 128],
                            qT[p][hl:hl + 64, qb * 512:(qb + 1) * 512],
                            start=True, stop=True)
                    pt = wrk.tile([128, 2, 512], f16, tag="pt", name="pt",
                                  bufs=3)
                    nc.scalar.activation(pt[:], sc[:], AF.Exp, scale=SCALE2)
                    if c2 >= 2 * qb:
                        dk = (c2 - 2 * qb) * 2
                        nc.vector.tensor_tensor(
                            pt[:], pt[:],
                            maskd_sb[:, qb * 4 + dk:qb * 4 + dk + 2, :],
                            op=OP.mult)
                    pts[c2] = pt
                    if c2 >= 1:
                        for jj in range(2):
                            nc.tensor.matmul(
                                av[:],
                                vaug[:, 2 * (c2 - 1) + jj, h, 0:HD + 1],
                                pts[c2 - 1][:, jj, :],
                                start=(c2 == 1 and jj == 0), stop=False)
                for jj in range(2):
                    nc.tensor.matmul(
                        av[:], vaug[:, 2 * (c2max - 1) + jj, h, 0:HD + 1],
                        pts[c2max - 1][:, jj, :],
                        start=(c2max == 1 and jj == 0), stop=(jj == 1))
                # stash numerator + denominator, freeing the psum quickly
                oraw[h] = wrk.tile([64, 512], f16, tag=f"oraw{h}",
                                   name=f"oraw{h}", bufs=2)
                nc.vector.tensor_copy(oraw[h][:], av[0:HD, :])
                nc.vector.tensor_copy(den[32 * h:32 * h + 1, :],
                                      av[HD:HD + 1, :])
            # one batched reciprocal per query block, then scale each head;
            # the broadcast is a K=1 matmul: ones[1,64].T @ deni_row[1,512]
            deni = wrk.tile([128, 512], f16, tag="deni", name="deni", bufs=2)
            with nc.allow_low_precision(reason="softmax denom recip in f16"):
                nc.vector.reciprocal(deni[:], den[:])
            for h in range(HPC):
                hl = (h % 2) * 64
                rb = psD.tile([HD + 1, 512], f32, tag="av", name="rbps",
                              bufs=2)
                nc.tensor.matmul(rb[0:HD, :], ones_sb[32 * h:32 * h + 1, :],
                                 deni[32 * h:32 * h + 1, :],
                                 start=True, stop=True,
                                 tile_position=(32 * h, 0))
                oT = wrk.tile([64, 512], f16, tag="oT", name="oT", bufs=2)
                nc.vector.tensor_tensor(oT[:], oraw[h][:], rb[0:HD, :],
                                        op=OP.mult)
                nc.sync.dma_start(o_sb[hl:hl + 64, h // 2, :], oT[:])

        def proj_rs(qb):
            # proj partials for this query block -> fp16 -> reduce-scatter
            o_sb = o_sbs.pop(qb)
            pj_sb = wrk.tile([128, DT, 4, 128], f16, tag="pj", name="pj",
                             bufs=1)
            for dc in range(DT):
                ps = psD.tile([128, 512], f32, tag="pj", name="pjps", bufs=1)
                for j in range(2):
                    nc.tensor.matmul(ps[:],
                                     wp_sb[:, j, dc * 128:(dc + 1) * 128],
                                     o_sb[:, j, :],
                                     start=(j == 0), stop=(j == 1))
                nc.vector.tensor_copy(pj_sb[:, dc, :, :], ps[:])
            for s in range(G):
                nc.sync.dma_start(
                    pjpart[qb][s * D:(s + 1) * D, :].rearrange(
                        "(c p) t -> p c t", p=128),
                    pj_sb[:, :, s, :])
            nc.gpsimd.collective_compute(
                "ReduceScatter", OP.add, replica_groups=groups,
                ins=[pjpart[qb][:].opt()], outs=[hpart[qb][:].opt()])

        def finish_strip(qb):
            """h = x + proj_rs^T for own strip of block qb."""
            hp = wrk.tile([128, DT, 128], f16, tag="hp", name="hp", bufs=2)
            nc.sync.dma_start(
                hp[:], hpart[qb][:].rearrange("(c p) t -> p c t", p=128))
            for dc in range(DT):
                ps = psD.tile([128, 128], f16, tag="tr", name="trD", bufs=1)
                nc.tensor.transpose(ps[:], hp[:, dc, :], identity[:])
                nc.vector.tensor_tensor(
                    h_sb[qb][:, dc * 128:(dc + 1) * 128], ps[:],
                    x_sb[qb][:, dc * 128:(dc + 1) * 128], op=OP.add)

        attend_heads(3)
        attend_heads(2)
        proj_rs(3)
        attend_heads(1)
        proj_rs(2)
        attend_heads(0)
        proj_rs(1)
        proj_rs(0)
        finish_strip(3)
        finish_strip(2)
        finish_strip(1)
        finish_strip(0)

        xnp.release()

        # ---------- phase G: LN2 -> yT ----------
        ynp = tc.alloc_tile_pool(name="ynp", bufs=1)
        yT = ynp.tile([128, DT, T], f16, tag="yT", name="yT")
        for i in (3, 2, 1, 0):
            yn = layer_norm_tile(h_sb[i], f"ln2_{i}")
            for dc in range(DT):
                ps = psD.tile([128, 128], f16, tag="tr", name="trG", bufs=1)
                nc.tensor.transpose(ps[:], yn[:, dc * 128:(dc + 1) * 128],
                                    identity[:])
                nc.vector.tensor_copy(yT[:, dc, i * 128:(i + 1) * 128], ps[:])
        attp.release()
        wqp.release()
        shp.release()
        xpool.release()
        psD.release()

        # ---------- phase H: ffn1 + gelu (fp16) ----------
        psH = tc.alloc_tile_pool(name="psH", bufs=2, space="PSUM")
        hidp = tc.alloc_tile_pool(name="hidp", bufs=1)
        hidT = hidp.tile([128, FT, T], f16, tag="hidT", name="hidT")
        for m in range(FT):
            ps = psH.tile([128, 512], f32, tag="f1", name="f1ps")
            for k in range(DT):
                nc.tensor.matmul(ps[:],
                                 w1_sb[:, k, m * 128:(m + 1) * 128],
                                 yT[:, k, :],
                                 start=(k == 0), stop=(k == DT - 1))
            nc.scalar.activation(hidT[:, m, :], ps[:], AF.Gelu)

        # ---------- phase I: ffn2 + residual -> out ----------
        w2p = tc.alloc_tile_pool(name="w2p", bufs=2)
        for nq in range(4):
            w2_sb = w2p.tile([128, FT, 256], f16, tag="w2", name="w2")
            for m in range(FT):
                nc.sync.dma_start(
                    w2_sb[:, m, :],
                    w2_in.ap()[m * 128:(m + 1) * 128,
                               nq * 256:(nq + 1) * 256])
            for i in range(4):
                ps = psH.tile([128, 256], f32, tag="f2", name="f2ps")
                for m in range(FT):
                    nc.tensor.matmul(ps[:],
                                     hidT[:, m, i * 128:(i + 1) * 128],
                                     w2_sb[:, m, :],
                                     start=(m == 0), stop=(m == FT - 1))
                o_sb = wrk.tile([128, 256], f32, tag="fo", name="fo")
                nc.vector.tensor_tensor(
                    o_sb[:], ps[:], h_sb[i][:, nq * 256:(nq + 1) * 256],
                    op=OP.add)
                nc.sync.dma_start(
                    out_t.ap()[i * 128:(i + 1) * 128,
                               nq * 256:(nq + 1) * 256], o_sb[:])
        psH.release()
        w2p.release()
        hidp.release()
        ynp.release()
        w1p.release()
        hxp.release()
        dp.release()
        wrk.release()
        pp.release()

    nc.compile()
    return nc


def _prep_fast(inputs):
    import ml_dtypes
    f8 = ml_dtypes.float8_e4m3

    x = np.asarray(inputs["x"], np.float32)
    mask = np.asarray(inputs["mask"])
    qkv_w = np.asarray(inputs["qkv_w"], np.float32)
    proj_w = np.asarray(inputs["proj_w"], np.float32)
    w1_16 = np.asarray(inputs["ffn_w1"], np.float32).astype(np.float16)
    w2_16 = np.asarray(inputs["ffn_w2"], np.float32).astype(np.float16)

    def to8(a):
        return np.ascontiguousarray(np.clip(a, -240, 240)).astype(f8)

    in_maps = []
    for r in range(N_CORES):
        b, j = r // G, r % G
        rows = np.concatenate(
            [np.arange(qb * 512 + j * 128, qb * 512 + j * 128 + 128)
             for qb in range(NQB)])
        wq = qkv_w[:, 256 * j: 256 * j + 256]
        wk = qkv_w[:, D + 256 * j: D + 256 * j + 256]
        wv = qkv_w[:, 2 * D + 256 * j: 2 * D + 256 * j + 256]
        maskd = np.concatenate(
            [(mask[b, 0, qb * 512:(qb + 1) * 512,
                   qb * 512:(qb + 1) * 512] != 0).T.astype(np.float32)
             for qb in range(NQB)], axis=0)
        im = {
            "x": np.ascontiguousarray(x[b, rows, :]),
            "maskd": maskd.astype(np.float16),
            "wqkv": to8(np.concatenate([wq, wk, wv], axis=1)),
            "wp": np.ascontiguousarray(
                proj_w[256 * j: 256 * j + 256, :]).astype(np.float16),
            "w_f1": w1_16, "w_f2": w2_16,
        }
        in_maps.append(im)
    return in_maps


def _gather_fast(res):
    out = np.empty((B, L, D), np.float32)
    for r in range(N_CORES):
        b, j = r // G, r % G
        o = res.results[r]["out"]
        for qb in range(NQB):
            out[b, qb * 512 + j * 128: qb * 512 + j * 128 + 128, :] = \
                o[qb * 128:(qb + 1) * 128, :]
    return out


def _fast_ok(inputs):
    """Fast path requires exact causal mask + trivial biases/affine."""
    mask = np.asarray(inputs["mask"])
    if mask.shape != (B, 1, L, L):
        return False
    tril = np.tril(np.ones((L, L), mask.dtype))
    for b in range(B):
        if not np.array_equal(mask[b, 0], tril):
            return False
    return (np.allclose(np.asarray(inputs["ln1_w"]), 1.0)
            and not np.any(np.asarray(inputs["ln1_b"]))
            and np.allclose(np.asarray(inputs["ln2_w"]), 1.0)
            and not np.any(np.asarray(inputs["ln2_b"]))
            and not np.any(np.asarray(inputs["qkv_b"]))
            and not np.any(np.asarray(inputs["proj_b"]))
            and not np.any(np.asarray(inputs["ffn_b1"]))
            and not np.any(np.asarray(inputs["ffn_b2"])))


# ------------------------------------------------------------------
# generic fallback: token-parallel kernel (previous version, verbatim)
# ------------------------------------------------------------------
def _build(cfg):
    import concourse.bass as bass
    from concourse import bacc, mybir
    import concourse.tile as tile
    from concourse.masks import make_identity

    f32 = mybir.dt.float32
    f16 = mybir.dt.float16
    AF = mybir.ActivationFunctionType
    OP = mybir.AluOpType

    TT = T // 128            # 4 token tiles
    DT = D // 128            # 8 dim chunks
    QKF = 2 * D              # q+k features
    KCg = L // 128           # 16 key chunks
    FT = FF // 128           # 32 ffn hidden chunks

    nc = bacc.Bacc("TRN2", target_bir_lowering=False, debug=False,
                   num_devices=N_CORES)

    x_in = nc.dram_tensor("x", [T, D], f32, kind="ExternalInput")
    mask_in = nc.dram_tensor("maskT", [L, T], f16, kind="ExternalInput")
    wqk_in = nc.dram_tensor("w_qk", [D, QKF], f16, kind="ExternalInput")
    wv_in = nc.dram_tensor("w_v", [D, D], f16, kind="ExternalInput")
    wp_in = nc.dram_tensor("w_proj", [D, D], f16, kind="ExternalInput")
    w1_in = nc.dram_tensor("w_f1", [D, FF], f16, kind="ExternalInput")
    w2_in = nc.dram_tensor("w_f2", [FF, D], f16, kind="ExternalInput")
    out_t = nc.dram_tensor("out", [T, D], f32, kind="ExternalOutput")

    opt = {}
    if cfg["ln1_affine"]:
        opt["ln1_wb"] = nc.dram_tensor("ln1_wb", [2, D], f32, kind="ExternalInput")
    if cfg["ln2_affine"]:
        opt["ln2_wb"] = nc.dram_tensor("ln2_wb", [2, D], f32, kind="ExternalInput")
    if cfg["qkv_bias"]:
        opt["bqk"] = nc.dram_tensor("bqk", [QKF, 1], f32, kind="ExternalInput")
        opt["bv"] = nc.dram_tensor("bv", [1, D], f32, kind="ExternalInput")
    if cfg["proj_bias"]:
        opt["bproj"] = nc.dram_tensor("bproj", [1, D], f32, kind="ExternalInput")
    if cfg["ffn1_bias"]:
        opt["bf1"] = nc.dram_tensor("bf1", [FF, 1], f32, kind="ExternalInput")
    if cfg["ffn2_bias"]:
        opt["bf2"] = nc.dram_tensor("bf2", [1, D], f32, kind="ExternalInput")

    with tile.TileContext(nc) as tc:
        pp = tc.alloc_tile_pool(name="persist", bufs=1)
        wp = tc.alloc_tile_pool(name="work", bufs=3)
        wtp = tc.alloc_tile_pool(name="wtile", bufs=6)
        pszoo = tc.alloc_tile_pool(name="psums", bufs=2, space="PSUM")
        dp = tc.alloc_tile_pool(name="dram", bufs=1, space="DRAM")

        identity = pp.tile([128, 128], f16, tag="identity", name="identity")
        make_identity(nc, identity[:])
        eps_sb = pp.tile([128, 1], f32, tag="eps", name="eps")
        nc.vector.memset(eps_sb[:], EPS)
        ones_sb = pp.tile([128, HD], f16, tag="ones", name="ones")
        nc.vector.memset(ones_sb[:], 1.0)

        def bcast_tile(src_ap, n, tag):
            row = pp.tile([1, n], f32, tag=tag + "r", name=tag + "r")
            nc.sync.dma_start(row[:], src_ap)
            t_ = pp.tile([128, n], f32, tag=tag, name=tag)
            nc.gpsimd.partition_broadcast(t_[:], row[:])
            return t_

        ln1_w_bc = ln1_b_bc = ln2_w_bc = ln2_b_bc = None
        if cfg["ln1_affine"]:
            ln1_w_bc = bcast_tile(opt["ln1_wb"].ap()[0:1, :], D, "ln1w")
            ln1_b_bc = bcast_tile(opt["ln1_wb"].ap()[1:2, :], D, "ln1b")
        if cfg["ln2_affine"]:
            ln2_w_bc = bcast_tile(opt["ln2_wb"].ap()[0:1, :], D, "ln2w")
            ln2_b_bc = bcast_tile(opt["ln2_wb"].ap()[1:2, :], D, "ln2b")
        bv_bc = bcast_tile(opt["bv"].ap(), D, "bv") if cfg["qkv_bias"] else None
        bp_bc = bcast_tile(opt["bproj"].ap(), D, "bp") if cfg["proj_bias"] else None
        bf2_bc = bcast_tile(opt["bf2"].ap(), D, "bf2") if cfg["ffn2_bias"] else None
        bqk_sb = None
        if cfg["qkv_bias"]:
            bqk_sb = [pp.tile([128, 1], f32, tag=f"bqk{f}", name=f"bqk{f}")
                      for f in range(QKF // 128)]
            for f in range(QKF // 128):
                nc.sync.dma_start(bqk_sb[f][:],
                                  opt["bqk"].ap()[f * 128:(f + 1) * 128, :])
        bf1_sb = None
        if cfg["ffn1_bias"]:
            bf1_sb = [pp.tile([128, 1], f32, tag=f"bf1{m}", name=f"bf1{m}")
                      for m in range(FT)]
            for m in range(FT):
                nc.sync.dma_start(bf1_sb[m][:],
                                  opt["bf1"].ap()[m * 128:(m + 1) * 128, :])

        def layer_norm_T(src_tiles, dstT_tiles, w_bc, b_bc, affine, tag):
            for i in range(TT):
                xt = src_tiles[i]
                mu = wp.tile([128, 1], f32, tag="lnmu", name="lnmu")
                nc.vector.tensor_reduce(mu[:], xt[:], mybir.AxisListType.X, OP.add)
                nc.vector.tensor_scalar_mul(mu[:], mu[:], 1.0 / D)
                junk = wp.tile([128, D], f16, tag="lnjunk", name="lnjunk", bufs=1)
                varr = wp.tile([128, 1], f32, tag="lnvar", name="lnvar")
                nc.vector.scalar_tensor_tensor(
                    junk[:], xt[:], mu[:], xt[:],
                    op0=OP.subtract, op1=OP.mult, accum_out=varr[:])
                std = wp.tile([128, 1], f32, tag="lnstd", name="lnstd")
                nc.scalar.activation(std[:], varr[:], AF.Sqrt,
                                     bias=eps_sb[:], scale=1.0 / D)
                rstd = wp.tile([128, 1], f32, tag="lnrstd", name="lnrstd")
                nc.vector.reciprocal(rstd[:], std[:])
                xn = wp.tile([128, D], f16, tag="lnxn", name="lnxn")
                nc.vector.tensor_scalar(xn[:], xt[:], mu[:], rstd[:],
                                        op0=OP.subtract, op1=OP.mult)
                if affine:
                    nc.vector.tensor_tensor(xn[:], xn[:], w_bc[:], op=OP.mult)
                    nc.vector.tensor_tensor(xn[:], xn[:], b_bc[:], op=OP.add)
                for j in range(DT):
                    ps = pszoo.tile([128, 128], f16, tag="tr", name="tr")
                    nc.tensor.transpose(ps[:], xn[:, j * 128:(j + 1) * 128],
                                        identity[:])
                    nc.vector.tensor_copy(
                        dstT_tiles[j][:, i * 128:(i + 1) * 128], ps[:])

        qkv_pool = tc.alloc_tile_pool(name="qkvp", bufs=1)
        xnT_pool = tc.alloc_tile_pool(name="xnT", bufs=1)
        wv_pool = tc.alloc_tile_pool(name="wvp", bufs=1)
        x_sb = [pp.tile([128, D], f32, tag=f"x{i}", name=f"x{i}")
                for i in range(TT)]
        for i in range(TT):
            nc.sync.dma_start(x_sb[i][:], x_in.ap()[i * 128:(i + 1) * 128, :])
        xnT = [xnT_pool.tile([128, T], f16, tag=f"xnT{j}", name=f"xnT{j}")
               for j in range(DT)]
        layer_norm_T(x_sb, xnT, ln1_w_bc, ln1_b_bc, cfg["ln1_affine"], "ln1")

        qkT = [qkv_pool.tile([128, T], f16, tag=f"qkT{f}", name=f"qkT{f}")
               for f in range(QKF // 128)]
        for f in range(QKF // 128):
            ps = pszoo.tile([128, T], f32, tag="acc", name="accqk")
            for j in range(DT):
                wt = wtp.tile([128, 128], f16, tag="wqk", name="wqk")
                nc.sync.dma_start(
                    wt[:], wqk_in.ap()[j * 128:(j + 1) * 128,
                                       f * 128:(f + 1) * 128])
                nc.tensor.matmul(ps[:], wt[:], xnT[j][:],
                                 start=(j == 0), stop=(j == DT - 1))
            if cfg["qkv_bias"]:
                nc.vector.tensor_scalar_add(qkT[f][:], ps[:], bqk_sb[f][:])
            else:
                nc.vector.tensor_copy(qkT[f][:], ps[:])

        qT_sb = [qkv_pool.tile([64, T], f16, tag=f"qT{h}", name=f"qT{h}")
                 for h in range(H)]
        for h in range(H):
            lo = (h % 2) * 64
            nc.sync.dma_start(qT_sb[h][:], qkT[h // 2][lo:lo + 64, :])

        wv_sb = [wv_pool.tile([128, 512], f16, tag=f"wv{k}", name=f"wv{k}")
                 for k in range(DT * 2)]
        for j in range(DT):
            for n in range(2):
                nc.sync.dma_start(
                    wv_sb[j * 2 + n][:],
                    wv_in.ap()[j * 128:(j + 1) * 128, n * 512:(n + 1) * 512])
        v_sb = [qkv_pool.tile([128, D], f16, tag=f"v{i}", name=f"v{i}")
                for i in range(TT)]
        for i in range(TT):
            for n in range(2):
                ps = pszoo.tile([128, 512], f32, tag="acc", name="accv")
                for j in range(DT):
                    nc.tensor.matmul(ps[:], xnT[j][:, i * 128:(i + 1) * 128],
                                     wv_sb[j * 2 + n][:],
                                     start=(j == 0), stop=(j == DT - 1))
                dst = v_sb[i][:, n * 512:(n + 1) * 512]
                if cfg["qkv_bias"]:
                    nc.vector.tensor_tensor(dst, ps[:],
                                            bv_bc[:, n * 512:(n + 1) * 512],
                                            op=OP.add)
                else:
                    nc.vector.tensor_copy(dst, ps[:])

        groups = [list(range(G)), list(range(G, 2 * G))]
        kt_shard = dp.tile([D, T], f16)
        v_shard = dp.tile([T, D], f16)
        for f in range(DT):
            nc.sync.dma_start(kt_shard[f * 128:(f + 1) * 128, :], qkT[DT + f][:])
        for i in range(TT):
            nc.sync.dma_start(v_shard[i * 128:(i + 1) * 128, :], v_sb[i][:])
        kt_g = dp.tile([G * D, T], f16)
        v_g = dp.tile([G * T, D], f16)
        nc.gpsimd.collective_compute(
            "AllGather", OP.bypass, replica_groups=groups,
            ins=[kt_shard[:].opt()], outs=[kt_g[:].opt()])
        nc.gpsimd.collective_compute(
            "AllGather", OP.bypass, replica_groups=groups,
            ins=[v_shard[:].opt()], outs=[v_g[:].opt()])
        wv_pool.release()
        xnT_pool.release()

        attn_pool = tc.alloc_tile_pool(name="attnp", bufs=1)
        mask_pool = tc.alloc_tile_pool(name="maskp", bufs=1)
        mask_sb = [mask_pool.tile([128, T], f16, tag=f"m{c}", name=f"m{c}")
                   for c in range(KCg)]
        for c in range(KCg):
            nc.sync.dma_start(mask_sb[c][:],
                              mask_in.ap()[c * 128:(c + 1) * 128, :])

        attnT = [attn_pool.tile([128, T], f16, tag=f"aT{j}", name=f"aT{j}")
                 for j in range(DT)]
        v_g_r = v_g[:].rearrange("(c p) n -> p c n", p=128)
        for h in range(H):
            kt_h = wp.tile([64, L], f16, tag="kt_h", name="kt_h", bufs=2)
            for g in range(G):
                nc.sync.dma_start(
                    kt_h[:, g * T:(g + 1) * T],
                    kt_g[g * D + h * HD:g * D + (h + 1) * HD, :])
            vaug = wp.tile([128, KCg, HD + 1], f16, tag="vaug", name="vaug",
                           bufs=2)
            nc.vector.memset(vaug[:, :, HD:HD + 1], 1.0)
            nc.sync.dma_start(vaug[:, :, 0:HD],
                              v_g_r[:, :, h * HD:(h + 1) * HD])

            o_ps = pszoo.tile([HD + 1, T], f32, tag="ops", name="ops")
            for c in range(KCg):
                s_ps = pszoo.tile([128, T], f32, tag="sps", name="sps")
                nc.tensor.matmul(s_ps[:], kt_h[:, c * 128:(c + 1) * 128],
                                 qT_sb[h][:], start=True, stop=True)
                pt = wp.tile([128, T], f16, tag="pt", name="pt")
                nc.scalar.activation(pt[:], s_ps[:], AF.Exp, scale=SCALE2)
                nc.vector.tensor_tensor(pt[:], pt[:], mask_sb[c][:], op=OP.mult)
                nc.tensor.matmul(o_ps[:], vaug[:, c:c + 1, :], pt[:],
                                 start=(c == 0), stop=(c == KCg - 1))
            recip = wp.tile([1, T], f32, tag="recip", name="recip", bufs=2)
            nc.vector.reciprocal(recip[:], o_ps[HD:HD + 1, :])
            rb = wp.tile([64, T], f32, tag="rb", name="rb", bufs=2)
            nc.gpsimd.partition_broadcast(rb[:], recip[:])
            oT_h = wp.tile([64, T], f16, tag="oT_h", name="oT_h", bufs=2)
            nc.vector.tensor_tensor(oT_h[:], o_ps[0:HD, :], rb[:], op=OP.mult)
            lo = (h % 2) * 64
            nc.sync.dma_start(attnT[h // 2][lo:lo + 64, :], oT_h[:])
        mask_pool.release()

        wpp = tc.alloc_tile_pool(name="wpp", bufs=1)
        wproj_sb = [wpp.tile([128, 512], f16, tag=f"wp{k}", name=f"wp{k}")
                    for k in range(DT * 2)]
        for j in range(DT):
            for n in range(2):
                nc.sync.dma_start(
                    wproj_sb[j * 2 + n][:],
                    wp_in.ap()[j * 128:(j + 1) * 128, n * 512:(n + 1) * 512])
        h_sb = [pp.tile([128, D], f32, tag=f"h{i}", name=f"h{i}")
                for i in range(TT)]
        for i in range(TT):
            for n in range(2):
                ps = pszoo.tile([128, 512], f32, tag="acc", name="accp")
                for j in range(DT):
                    nc.tensor.matmul(ps[:], attnT[j][:, i * 128:(i + 1) * 128],
                                     wproj_sb[j * 2 + n][:],
                                     start=(j == 0), stop=(j == DT - 1))
                dst = h_sb[i][:, n * 512:(n + 1) * 512]
                xsl = x_sb[i][:, n * 512:(n + 1) * 512]
                if cfg["proj_bias"]:
                    nc.vector.tensor_tensor(dst, ps[:],
                                            bp_bc[:, n * 512:(n + 1) * 512],
                                            op=OP.add)
                    nc.vector.tensor_tensor(dst, dst, xsl, op=OP.add)
                else:
                    nc.vector.tensor_tensor(dst, ps[:], xsl, op=OP.add)
        wpp.release()
        attn_pool.release()
        qkv_pool.release()

        hid_pool = tc.alloc_tile_pool(name="hidp", bufs=1)
        yT_pool = tc.alloc_tile_pool(name="yTp", bufs=1)
        yT = [yT_pool.tile([128, T], f16, tag=f"yT{j}", name=f"yT{j}")
              for j in range(DT)]
        layer_norm_T(h_sb, yT, ln2_w_bc, ln2_b_bc, cfg["ln2_affine"], "ln2")

        hidT = [hid_pool.tile([128, T], f16, tag=f"hidT{m}", name=f"hidT{m}")
                for m in range(FT)]
        for m in range(FT):
            ps = pszoo.tile([128, T], f32, tag="acc", name="accf1")
            for j in range(DT):
                wt = wtp.tile([128, 128], f16, tag="w1", name="w1")
                nc.sync.dma_start(
                    wt[:], w1_in.ap()[j * 128:(j + 1) * 128,
                                      m * 128:(m + 1) * 128])
                nc.tensor.matmul(ps[:], wt[:], yT[j][:],
                                 start=(j == 0), stop=(j == DT - 1))
            if cfg["ffn1_bias"]:
                nc.scalar.activation(hidT[m][:], ps[:], AF.Gelu,
                                     bias=bf1_sb[m][:])
            else:
                nc.scalar.activation(hidT[m][:], ps[:], AF.Gelu)
        yT_pool.release()

        w2_pool = tc.alloc_tile_pool(name="w2p", bufs=1)
        for n in range(2):
            w2_sb = [w2_pool.tile([128, 512], f16, tag=f"w2_{m}",
                                  name=f"w2_{m}") for m in range(FT)]
            for m in range(FT):
                nc.sync.dma_start(
                    w2_sb[m][:], w2_in.ap()[m * 128:(m + 1) * 128,
                                            n * 512:(n + 1) * 512])
            for i in range(TT):
                ps = pszoo.tile([128, 512], f32, tag="acc", name="accf2")
                for m in range(FT):
                    nc.tensor.matmul(ps[:], hidT[m][:, i * 128:(i + 1) * 128],
                                     w2_sb[m][:],
                                     start=(m == 0), stop=(m == FT - 1))
                o_sb = wp.tile([128, 512], f32, tag="o_sb", name="o_sb")
                hsl = h_sb[i][:, n * 512:(n + 1) * 512]
                if cfg["ffn2_bias"]:
                    nc.vector.tensor_tensor(o_sb[:], ps[:],
                                            bf2_bc[:, n * 512:(n + 1) * 512],
                                            op=OP.add)
                    nc.vector.tensor_tensor(o_sb[:], o_sb[:], hsl, op=OP.add)
                else:
                    nc.vector.tensor_tensor(o_sb[:], ps[:], hsl, op=OP.add)
                nc.sync.dma_start(
                    out_t.ap()[i * 128:(i + 1) * 128,
                               n * 512:(n + 1) * 512], o_sb[:])
        w2_pool.release()
        hid_pool.release()
        wtp.release()
        wp.release()
        pszoo.release()
        dp.release()
        pp.release()

    nc.compile()
    return nc


def _prep(inputs):
    x = np.asarray(inputs["x"], np.float32)
    mask = np.asarray(inputs["mask"])
    qkv_w = np.asarray(inputs["qkv_w"], np.float32)
    qkv_b = np.asarray(inputs["qkv_b"], np.float32)
    proj_w = np.asarray(inputs["proj_w"], np.float32)
    proj_b = np.asarray(inputs["proj_b"], np.float32)
    ffn_w1 = np.asarray(inputs["ffn_w1"], np.float32)
    ffn_b1 = np.asarray(inputs["ffn_b1"], np.float32)
    ffn_w2 = np.asarray(inputs["ffn_w2"], np.float32)
    ffn_b2 = np.asarray(inputs["ffn_b2"], np.float32)
    ln1_w = np.asarray(inputs["ln1_w"], np.float32)
    ln1_b = np.asarray(inputs["ln1_b"], np.float32)
    ln2_w = np.asarray(inputs["ln2_w"], np.float32)
    ln2_b = np.asarray(inputs["ln2_b"], np.float32)

    cfg = {
        "ln1_affine": not (np.allclose(ln1_w, 1.0) and np.allclose(ln1_b, 0.0)),
        "ln2_affine": not (np.allclose(ln2_w, 1.0) and np.allclose(ln2_b, 0.0)),
        "qkv_bias": bool(np.any(qkv_b)),
        "proj_bias": bool(np.any(proj_b)),
        "ffn1_bias": bool(np.any(ffn_b1)),
        "ffn2_bias": bool(np.any(ffn_b2)),
    }

    w_qk = np.ascontiguousarray(qkv_w[:, :2 * D]).astype(np.float16)
    w_v = np.ascontiguousarray(qkv_w[:, 2 * D:]).astype(np.float16)
    w_p16 = proj_w.astype(np.float16)
    w1_16 = ffn_w1.astype(np.float16)
    w2_16 = ffn_w2.astype(np.float16)

    in_maps = []
    for r in range(N_CORES):
        b = r // G
        row0 = (r % G) * T
        im = {
            "x": np.ascontiguousarray(x[b, row0:row0 + T, :]),
            "maskT": np.ascontiguousarray(
                (mask[b, 0, row0:row0 + T, :] != 0).T.astype(np.float16)),
            "w_qk": w_qk, "w_v": w_v, "w_proj": w_p16,
            "w_f1": w1_16, "w_f2": w2_16,
        }
        if cfg["ln1_affine"]:
            im["ln1_wb"] = np.ascontiguousarray(np.stack([ln1_w, ln1_b]))
        if cfg["ln2_affine"]:
            im["ln2_wb"] = np.ascontiguousarray(np.stack([ln2_w, ln2_b]))
        if cfg["qkv_bias"]:
            im["bqk"] = np.ascontiguousarray(qkv_b[:2 * D, None])
            im["bv"] = np.ascontiguousarray(qkv_b[None, 2 * D:])
        if cfg["proj_bias"]:
            im["bproj"] = np.ascontiguousarray(proj_b[None, :])
        if cfg["ffn1_bias"]:
            im["bf1"] = np.ascontiguousarray(ffn_b1[:, None])
        if cfg["ffn2_bias"]:
            im["bf2"] = np.ascontiguousarray(ffn_b2[None, :])
        in_maps.append(im)
    return cfg, in_maps


def _run(inputs, trace=False):
    from concourse.bass_utils import run_bass_kernel_spmd

    if _fast_ok(inputs):
        if "fast" not in _cache:
            _cache["fast"] = _build_fast()
        nc = _cache["fast"]
        in_maps = _prep_fast(inputs)
        res = run_bass_kernel_spmd(nc, in_maps, core_ids=list(range(N_CORES)),
                                   trace=trace)
        return _gather_fast(res), res

    cfg, in_maps = _prep(inputs)
    key = tuple(sorted(cfg.items()))
    if key not in _cache:
        _cache[key] = _build(cfg)
    nc = _cache[key]
    res = run_bass_kernel_spmd(nc, in_maps, core_ids=list(range(N_CORES)),
                               trace=trace)
    out = np.empty((B, L, D), np.float32)
    for r in range(N_CORES):
        b = r // G
        row0 = (r % G) * T
        out[b, row0:row0 + T, :] = res.results[r]["out"]
    return out, res


def kernel(**inputs):
    out, _ = _run(inputs, trace=False)
    return out
